# revision 4
# baseline (speedup 1.0000x reference)
"""GCN (3-layer + mean-pool head) on 8 Trainium2 cores — v3 = v2 + bf16 L2 path.

bf16: h1 table (halves L2 gather traffic + h1 AllGather), L2 msg/ind matmuls
(4x PE stream rate vs fp32), C matrix + q (halves head DMA). L1 stays fp32.
"""

_V2_DOC = """GCN (3-layer + mean-pool head) on 8 Trainium2 cores — v2, slim inputs.

Differences from v1:
  - ind matrices built ON DEVICE from per-slot (wcol, norm) arrays via
    iota-ramp is_equal + multiply (upload 1.2MB/layer instead of 30MB).
  - idx uploaded un-tiled [16, TOT*8] and replicated to 128 partitions on
    device (0.3MB instead of 2.4MB per layer).
  - x uploaded sharded [NPC, FIN] per core and AllGathered on device
    (1.6MB instead of 12.8MB per core).
Per-group block enumeration: gather order per half = fulls then tails;
ind columns: fulls region (width 32 each) then tails region (width 128).
"""  # noqa: E501

from dataclasses import dataclass
import numpy as np

import concourse.bass as bass
import concourse.bacc as bacc
import concourse.mybir as mybir
import concourse.tile as tile
from concourse import ap_utils
from concourse._compat import exact_div
from concourse.masks import make_identity


def dma_gather_raw(gp, out_ap, in_ap, idxs_ap, num_idxs, num_idxs_reg, elem_size,
                   elem_step, single_packet=False):
    """BassGpSimd.dma_gather with the elem-size assert relaxed to 128B.

    The ISA encodes the table ROW STRIDE in 256-byte units
    (stride_bytes_256); the gathered element itself may be 128B —
    verified correct on HW (bench_elem128.py). Lets L1 gather 64 bf16
    features from a [N, 128] bf16 (256B-stride) table."""
    assert idxs_ap.dtype == mybir.dt.int16
    assert in_ap.space == bass.MemorySpace.DRAM
    assert idxs_ap.space == bass.MemorySpace.SBUF
    assert out_ap.space == bass.MemorySpace.SBUF
    assert in_ap.dtype == out_ap.dtype
    elem_size_bytes = elem_size * mybir.dt.size(in_ap.dtype)
    assert elem_size_bytes % 128 == 0
    assert ap_utils.ap_is_contiguous(in_ap.ap[1:])
    assert ap_utils.ap_is_contiguous(out_ap.ap[1:])
    assert ap_utils.ap_is_contiguous(idxs_ap.ap[1:])
    assert in_ap.ap[-1][1] == out_ap.ap[-1][1] == elem_size
    assert out_ap.ap[0][1] * out_ap.ap[1][1] == -(-num_idxs // 128) * 128
    assert in_ap.ap[0][0] == elem_step
    stride_bytes = elem_step * mybir.dt.size(in_ap.dtype)
    stride_bytes_256 = exact_div(stride_bytes, 256)
    assert stride_bytes_256 < 256
    _in_ap = gp.lower_ap_dma(in_ap, for_custom_bir_dma=True)
    _idxs_ap = gp.lower_ap(idxs_ap)
    _out_ap = gp.lower_ap(out_ap)
    return gp.add_instruction(
        mybir.InstDMAGatherAnt(
            name=gp.bass.get_next_instruction_name(),
            ins=[*_in_ap, _idxs_ap, gp.lower_val_access(gp.to_reg(num_idxs_reg))],
            outs=[_out_ap],
            transpose=False,
            num_idxs=num_idxs,
            elem_size=elem_size,
            stride_bytes_256=stride_bytes_256,
            gen_mode=0,
            single_packet=single_packet,
            queue_num=0,
            sbuf_tokens_per_rank=0,
            sbuf_free_dim_per_rank=0,
            sbuf_free_dim_pad_per_rank=0,
            sbuf_byte_offset=0,
        )
    )

BLK = 128
W = 32
NW = 4


@dataclass
class Cfg:
    N: int = 50000
    E: int = 1000000
    G: int = 128
    FIN: int = 64
    H: int = 128
    H2: int = 256
    NC: int = 8
    CG: int = 4
    SPLIT: int = 32768

    @property
    def NPC(self):
        assert self.N % self.NC == 0
        return self.N // self.NC

    @property
    def CH(self):
        return (self.NPC + 127) // 128

    @property
    def PADN(self):
        return self.CH * 128

    @property
    def NG(self):
        return (self.CH + self.CG - 1) // self.CG


def _ceil_div(a, b):
    return -(-a // b)


class LayerStruct:
    """Block structure shared across cores + per-core compact arrays.

    Per group g (CG chunks):
      gather order: half h: [fulls(k asc, j asc, b), tails(k asc, b)] -> cs
      ind columns:  fulls region [fulls h0 ++ fulls h1] (width 32 each),
                    tails region [tails h0 ++ tails h1] (width 128 each)
      idx16 columns: group base gcol0 = first_blk*8; h0 blocks then h1 blocks
        in gather order, 8 int16 cols per block.
    Per-core arrays:
      idx16 [16, TOT*8]   wrapped gather indices (block-major in gather order)
      wcol  [128, TOT_ind] f32 window col per slot (ind order: per group fulls
                           then tails, concatenated over groups)
      nval  [128, TOT_ind] f32 norm per slot (0 padding)
    """

    def __init__(self, cfg: Cfg, rows, dst, norm, n_table_rows):
        NC, CH, NPC, CG, SPLIT = cfg.NC, cfg.CH, cfg.NPC, cfg.CG, cfg.SPLIT
        core = dst // NPC
        l = dst - core * NPC
        k = l >> 7
        j = (l >> 5) & 3
        w32 = l & 31
        w128 = l & 127
        half = (rows >= SPLIT).astype(np.int64)
        self.n_lo_rows = min(SPLIT, n_table_rows)
        self.n_hi_rows = max(0, n_table_rows - SPLIT)

        key = (((core * CH + k) * 2 + half) * NW + j)
        counts = np.bincount(key, minlength=NC * CH * 2 * NW).reshape(NC, CH, 2, NW)
        Bfull = (counts // BLK).max(axis=0)  # [CH, 2, NW]
        leftover = counts - np.minimum(counts, Bfull[None] * BLK)
        tail_cnt = leftover.sum(axis=3)  # [NC, CH, 2]
        Btail = _ceil_div(tail_cnt, BLK).max(axis=0)  # [CH, 2]
        self.Bfull, self.Btail = Bfull, Btail

        # --- enumerate blocks ---
        # per (k,h,j): gather cs base; per (k,h): tail cs base
        # per block: ind column offset (fulls then tails region per group)
        full_cs = np.zeros((CH, 2, NW), dtype=np.int64)  # cs of first full blk
        tail_cs = np.zeros((CH, 2), dtype=np.int64)
        full_sg = np.zeros((CH, 2, NW), dtype=np.int64)  # global gather slot base
        tail_sg = np.zeros((CH, 2), dtype=np.int64)
        full_ic = np.zeros((CH, 2, NW), dtype=np.int64)  # ind col offset (global)
        tail_ic = np.zeros((CH, 2), dtype=np.int64)
        # ind-order column index (into wcol/nval [*, TOT_ind]) per block
        full_bc = np.zeros((CH, 2, NW), dtype=np.int64)
        tail_bc = np.zeros((CH, 2), dtype=np.int64)

        self.groups = []
        self.chunk_blocks = [None] * CH  # list of (h, cs, ric_kind, roff, width, ooff)
        cur_blk = 0  # global block counter (gather order, h-grouped per group)
        cur_ic = 0  # global ind col counter
        cur_bc = 0  # global ind-order block col counter
        for g in range(cfg.NG):
            ks = list(range(g * CG, min((g + 1) * CG, CH)))
            first_blk = cur_blk
            # gather order per half
            half_cnt = [0, 0]
            for h in (0, 1):
                cs = 0
                for kk in ks:
                    for jj in range(NW):
                        full_cs[kk, h, jj] = cs
                        cs += Bfull[kk, h, jj]
                for kk in ks:
                    tail_cs[kk, h] = cs
                    cs += Btail[kk, h]
                half_cnt[h] = cs
            nlo, nhi = half_cnt
            for h in (0, 1):
                base = first_blk + (nlo if h else 0)
                for kk in ks:
                    for jj in range(NW):
                        full_sg[kk, h, jj] = (base + full_cs[kk, h, jj]) * BLK
                    tail_sg[kk, h] = (base + tail_cs[kk, h]) * BLK
            # ind columns: fulls h0 ++ fulls h1, then tails h0 ++ tails h1
            first_ic = cur_ic
            first_bc = cur_bc
            nf = 0
            for h in (0, 1):
                for kk in ks:
                    for jj in range(NW):
                        full_ic[kk, h, jj] = cur_ic
                        full_bc[kk, h, jj] = cur_bc
                        cur_ic += Bfull[kk, h, jj] * W
                        cur_bc += Bfull[kk, h, jj]
                        nf += Bfull[kk, h, jj]
            ic_tail0 = cur_ic
            bc_tail0 = cur_bc
            nt = 0
            for h in (0, 1):
                for kk in ks:
                    tail_ic[kk, h] = cur_ic
                    tail_bc[kk, h] = cur_bc
                    cur_ic += Btail[kk, h] * BLK
                    cur_bc += Btail[kk, h]
                    nt += Btail[kk, h]
            cur_blk += nlo + nhi
            self.groups.append(
                dict(
                    chunks=ks,
                    first_blk=first_blk,
                    lo_cnt=nlo,
                    hi_cnt=nhi,
                    nf=nf,
                    nt=nt,
                    first_ic=first_ic,  # fulls ind region start (global col)
                    tail_ic0=ic_tail0,  # tails ind region start
                    first_bc=first_bc,  # fulls block-col start in wcol/nval
                    tail_bc0=bc_tail0,
                )
            )
            # per-chunk emission metadata
            for kk in ks:
                bl = []
                for h in (0, 1):
                    for jj in range(NW):
                        for b in range(Bfull[kk, h, jj]):
                            cs = full_cs[kk, h, jj] + b
                            ric = full_ic[kk, h, jj] + b * W - first_ic
                            bl.append((h, cs, "full", ric, W, jj * W))
                    for b in range(Btail[kk, h]):
                        cs = tail_cs[kk, h] + b
                        ric = tail_ic[kk, h] + b * BLK - ic_tail0
                        bl.append((h, cs, "tail", ric, BLK, 0))
                self.chunk_blocks[kk] = bl
        self.TOT = cur_blk
        self.IND_COLS = cur_ic
        self.TOTB = cur_bc  # == TOT

        # --- vectorized edge -> (slot, block) assignment ---
        order = np.lexsort((j, key))
        sk = key[order]
        newgrp = np.ones(len(sk), dtype=bool)
        newgrp[1:] = sk[1:] != sk[:-1]
        starts = np.flatnonzero(newgrp)
        lengths = np.diff(np.append(starts, len(sk)))
        rank_sorted = np.arange(len(sk)) - np.repeat(starts, lengths)
        rank = np.empty(len(sk), dtype=np.int64)
        rank[order] = rank_sorted  # rank within (core,k,half,j)

        capacity = Bfull[k, half, j] * BLK
        is_full = rank < capacity
        lo_pref = np.cumsum(leftover, axis=3) - leftover
        tail_rank = lo_pref[core, k, half, j] + (rank - capacity)

        # gather slot (s_global into idx16)
        sg_full = full_sg[k, half, j] + rank
        sg_tail = tail_sg[k, half] + tail_rank
        sg = np.where(is_full, sg_full, sg_tail)
        slot = np.where(is_full, rank % BLK, tail_rank % BLK)
        # ind-order block col (into wcol/nval) and window col
        bc_full = full_bc[k, half, j] + rank // BLK
        bc_tail_ = tail_bc[k, half] + tail_rank // BLK
        bc = np.where(is_full, bc_full, bc_tail_)
        wc = np.where(is_full, w32, w128)

        self.per_core = []
        for c in range(NC):
            m = core == c
            ncols = self.TOT * BLK // 16
            idx16 = np.zeros((16, ncols), dtype=np.int16)
            sgm = sg[m]
            vals = (rows[m] - half[m] * SPLIT).astype(np.int16)
            idx16[sgm % 16, sgm // 16] = vals
            wcol = np.zeros((BLK, self.TOTB), dtype=np.float32)
            nval = np.zeros((BLK, self.TOTB), dtype=np.float32)
            wcol[slot[m], bc[m]] = wc[m].astype(np.float32)
            nval[slot[m], bc[m]] = norm[m]
            self.per_core.append((idx16, wcol, nval))


def preprocess(cfg: Cfg, inputs):
    x = np.asarray(inputs["x"], dtype=np.float32)
    ei = np.asarray(inputs["edge_index"], dtype=np.int64)
    batch = np.asarray(inputs["batch"], dtype=np.int64)
    W1 = np.asarray(inputs["W1"], np.float32)
    b1 = np.asarray(inputs["b1"], np.float32)
    W2 = np.asarray(inputs["W2"], np.float32)
    b2 = np.asarray(inputs["b2"], np.float32)
    W3 = np.asarray(inputs["W3"], np.float32)
    b3 = np.asarray(inputs["b3"], np.float32)
    linW = np.asarray(inputs["linW"], np.float32)
    linb = np.asarray(inputs["linb"], np.float32)

    N, NC, NPC, PADN, CH, G = cfg.N, cfg.NC, cfg.NPC, cfg.PADN, cfg.CH, cfg.G
    src = np.concatenate([ei[0], np.arange(N, dtype=np.int64)])
    dst = np.concatenate([ei[1], np.arange(N, dtype=np.int64)])
    deg = np.bincount(dst, minlength=N).astype(np.float32)
    dinv = 1.0 / np.sqrt(deg)
    norm = (dinv[src] * dinv[dst]).astype(np.float32)

    L1 = LayerStruct(cfg, src, dst, norm, n_table_rows=N)
    r_of = (src // NPC) * PADN + (src % NPC)
    L2 = LayerStruct(cfg, r_of, dst, norm, n_table_rows=NC * PADN)

    cnt = np.maximum(np.bincount(batch, minlength=G), 1).astype(np.float32)
    coef = norm / cnt[batch[dst]]
    c_src = src // NPC
    kk = (src % NPC) >> 7
    ll = (src % NPC) & 127
    gg = batch[dst]
    flat = ((c_src * CH + kk) * 128 + ll) * G + gg
    C = np.bincount(flat, weights=coef.astype(np.float64), minlength=NC * CH * 128 * G)
    C = C.reshape(NC, CH * 128, G).astype(np.float32)

    w3 = (W3 @ linW).astype(np.float32)
    c_const = float(b3 @ linW[:, 0] + linb[0])
    empty = np.bincount(batch, minlength=G) == 0

    H = cfg.H
    bfnp = mybir.dt.np(mybir.dt.bfloat16)
    in_maps = []
    for c in range(NC):
        idx1, wcol1, nval1 = L1.per_core[c]
        idx2, wcol2, nval2 = L2.per_core[c]
        in_maps.append(
            {
                "xsh": x[c * NPC : (c + 1) * NPC, :].copy(),
                "W1": W1,
                "b1": b1.reshape(H, 1),
                "W2": W2,
                "b2": b2.reshape(2, H).T.copy(),
                "w3": w3.reshape(2, H).T.copy(),
                "idx1": idx1,
                "wcol1": wcol1.astype(bfnp),
                "nval1": nval1.astype(bfnp),
                "idx2": idx2,
                "wcol2": wcol2.astype(bfnp),
                "nval2": nval2.astype(bfnp),
                "C": C[c].astype(bfnp),
            }
        )
    host = dict(c_const=c_const, empty=empty, linb=float(linb[0]))
    return L1, L2, in_maps, host


def build_module(cfg: Cfg, L1: LayerStruct, L2: LayerStruct, single_core: bool = False, probe: str = ""):
    N, NC, NPC, PADN, CH, G = cfg.N, cfg.NC, cfg.NPC, cfg.PADN, cfg.CH, cfg.G
    FIN, H, H2 = cfg.FIN, cfg.H, cfg.H2
    f32 = mybir.dt.float32
    bf16 = mybir.dt.bfloat16
    i16 = mybir.dt.int16

    nc = bacc.Bacc("TRN2", debug=False, num_devices=1 if single_core else NC)
    xsh_t = nc.dram_tensor("xsh", [NPC, FIN], f32, kind="ExternalInput")
    W1_t = nc.dram_tensor("W1", [FIN, H], f32, kind="ExternalInput")
    b1_t = nc.dram_tensor("b1", [H, 1], f32, kind="ExternalInput")
    W2_t = nc.dram_tensor("W2", [H, H2], f32, kind="ExternalInput")
    b2_t = nc.dram_tensor("b2", [H, 2], f32, kind="ExternalInput")
    w3_t = nc.dram_tensor("w3", [H, 2], f32, kind="ExternalInput")
    idx1_t = nc.dram_tensor("idx1", [16, L1.TOT * 8], i16, kind="ExternalInput")
    wcol1_t = nc.dram_tensor("wcol1", [128, L1.TOTB], bf16, kind="ExternalInput")
    nval1_t = nc.dram_tensor("nval1", [128, L1.TOTB], bf16, kind="ExternalInput")
    idx2_t = nc.dram_tensor("idx2", [16, L2.TOT * 8], i16, kind="ExternalInput")
    wcol2_t = nc.dram_tensor("wcol2", [128, L2.TOTB], bf16, kind="ExternalInput")
    nval2_t = nc.dram_tensor("nval2", [128, L2.TOTB], bf16, kind="ExternalInput")
    C_t = nc.dram_tensor("C", [CH * 128, G], bf16, kind="ExternalInput")
    out_t = nc.dram_tensor("out", [G, 1], f32, kind="ExternalOutput")

    # bf16 x table padded to 128 cols (gather elem must be a multiple of 256B)
    xbfloc = nc.dram_tensor("xbfloc", [NPC, 128], bf16)
    xbffull = nc.dram_tensor("xbffull", [N, 128], bf16, addr_space="Shared")
    h1sh = nc.dram_tensor("h1sh", [PADN, H], bf16)
    h1full = nc.dram_tensor("h1full", [NC * PADN, H], bf16, addr_space="Shared")

    with tile.TileContext(nc) as tc:
        with (
            tc.tile_pool(name="const", bufs=1) as cpool,
            tc.tile_pool(name="idxall", bufs=1) as idxallp,
            tc.tile_pool(name="slotd", bufs=1) as slotp,
            tc.tile_pool(name="ind", bufs=2) as indp,
            tc.tile_pool(name="msg", bufs=2) as msgp,
            tc.tile_pool(name="sb", bufs=2) as sbp,
            tc.tile_pool(name="qpool", bufs=1) as qpool,
            tc.tile_pool(name="zps", bufs=2, space="PSUM") as zpsp,
            tc.tile_pool(name="hps", bufs=2, space="PSUM") as hpsp,
            tc.tile_pool(name="tps", bufs=1, space="PSUM") as tpsp,
            tc.tile_pool(name="qps", bufs=1, space="PSUM") as qpsp,
            tc.tile_pool(name="pps", bufs=1, space="PSUM") as ppsp,
            tc.tile_pool(name="scr", bufs=1, space="PSUM") as scrp,
        ):
            zero_sb = cpool.tile([128, 128], f32)
            nc.vector.memset(zero_sb[:], 0.0)
            zero_bf = cpool.tile([128, 128], bf16)
            nc.vector.memset(zero_bf[:], 0.0)
            ident = cpool.tile([128, 128], f32)
            make_identity(nc, ident[:])
            W1_sb = cpool.tile([FIN, H], f32)
            nc.sync.dma_start(out=W1_sb[:], in_=W1_t[:, :])
            b1_sb = cpool.tile([H, 1], f32)
            nc.sync.dma_start(out=b1_sb[:], in_=b1_t[:, :])
            W2_sb = cpool.tile([H, H2], f32)
            nc.sync.dma_start(out=W2_sb[:], in_=W2_t[:, :])
            b2_sb = cpool.tile([H, 2], f32)
            nc.sync.dma_start(out=b2_sb[:], in_=b2_t[:, :])
            w3_sb = cpool.tile([H, 2], f32)
            nc.sync.dma_start(out=w3_sb[:], in_=w3_t[:, :])
            scr_ps = scrp.tile([1, 1], f32, space="PSUM")
            q_sb = qpool.tile([128, CH], bf16)
            pool_ps = ppsp.tile([G, 1], f32, space="PSUM")
            # whole C matrix resident: one DMA instead of 49 small ones
            Call = qpool.tile([128, CH * G], bf16, name="Call")
            nc.sync.dma_start(
                out=Call[:].rearrange("p (k g) -> p k g", k=CH),
                in_=C_t[:, :].rearrange("(k p) g -> p k g", k=CH),
            )

            # iota ramps 0..31 and 0..127 (f32, same value in every partition)
            ramps = {}
            for dt_, dname in ((f32, "f"), (bf16, "b")):
                r32 = cpool.tile([128, W], dt_, name=f"ramp32{dname}")
                nc.gpsimd.iota(
                    r32[:], [[1, W]], channel_multiplier=0,
                    allow_small_or_imprecise_dtypes=True,
                )
                r128 = cpool.tile([128, BLK], dt_, name=f"ramp128{dname}")
                nc.gpsimd.iota(
                    r128[:], [[1, BLK]], channel_multiplier=0,
                    allow_small_or_imprecise_dtypes=True,
                )
                ramps[dt_] = (r32, r128)

            def absorb(dep_ap):
                kdim = dep_ap.shape[0]
                zt = zero_bf if dep_ap.dtype == bf16 else zero_sb
                nc.tensor.matmul(
                    scr_ps[:], lhsT=zt[:kdim, :1], rhs=dep_ap, start=True, stop=True
                )

            absorb(zero_sb[:, :1])
            for cst in (ident, W1_sb, b1_sb, W2_sb, b2_sb, w3_sb):
                absorb(cst[:, :1])
            act_scr = cpool.tile([H, 3], f32)
            nc.scalar.copy(act_scr[:, 0:1], b1_sb[:, :1])
            nc.scalar.copy(act_scr[:, 1:2], b2_sb[:, 0:1])
            nc.scalar.copy(act_scr[:, 2:3], b2_sb[:, 1:2])
            absorb(Call[:, :1])

            # ---- convert own x shard to bf16 padded [NPC, 128], AllGather ----
            # (collectives cannot read IO tensors, so the staging through
            # xbfloc also satisfies that rule)
            # bulk part: 6144 rows = 128 partitions x 48 rows, contiguous per
            # partition on both DRAM sides -> two large efficient DMAs
            RPP = NPC // 128  # 48 rows per partition
            NB = RPP * 128
            xs_cv = cpool.tile([128, RPP * FIN], f32, name="xs_cv")
            nc.sync.dma_start(
                out=xs_cv[:].rearrange("p (r f) -> p r f", f=FIN),
                in_=xsh_t[0:NB, :].rearrange("(p r) f -> p r f", r=RPP),
            )
            xcv = cpool.tile([128, RPP * 128], bf16, name="xcv")
            nc.vector.memset(xcv[:], 0.0)
            nc.vector.tensor_copy(
                out=xcv[:].rearrange("p (r k) -> p r k", k=128)[:, :, 0:FIN],
                in_=xs_cv[:].rearrange("p (r f) -> p r f", f=FIN),
            )
            nc.sync.dma_start(
                out=xbfloc[0:NB, :].rearrange("(p r) k -> p r k", r=RPP),
                in_=xcv[:].rearrange("p (r k) -> p r k", k=128),
            )
            # tail rows
            ntail = NPC - NB
            if ntail:
                xst = cpool.tile([128, FIN], f32, name="xst")
                nc.sync.dma_start(out=xst[:ntail, :], in_=xsh_t[NB:NPC, :])
                xcvt = cpool.tile([128, 128], bf16, name="xcvt")
                nc.vector.memset(xcvt[:], 0.0)
                nc.vector.tensor_copy(out=xcvt[:ntail, :FIN], in_=xst[:ntail, :])
                nc.sync.dma_start(out=xbfloc[NB:NPC, :], in_=xcvt[:ntail, :])
            if single_core:
                nc.sync.dma_start(out=xbffull[0:NPC, :], in_=xbfloc[:, :])
            else:
                nc.gpsimd.collective_compute(
                    "AllGather",
                    mybir.AluOpType.bypass,
                    replica_groups=[list(range(NC))],
                    ins=[xbfloc[:, :]],
                    outs=[xbffull[:, :]],
                )

            # ---- per-layer slot data ----
            def load_layer_inputs(LS, idx_t, wcol_t, nval_t, tag, dt_):
                idx_sb = idxallp.tile([128, LS.TOT * 8], i16, tag=f"idx{tag}")
                for r in range(8):
                    nc.sync.dma_start(
                        out=idx_sb[16 * r : 16 * (r + 1), :], in_=idx_t[:, :]
                    )
                wcol_sb = slotp.tile([128, LS.TOTB], dt_, tag=f"wc{tag}")
                nc.sync.dma_start(out=wcol_sb[:], in_=wcol_t[:, :])
                nval_sb = slotp.tile([128, LS.TOTB], dt_, tag=f"nv{tag}")
                nc.sync.dma_start(out=nval_sb[:], in_=nval_t[:, :])
                return idx_sb, wcol_sb, nval_sb

            def sparse_layer(LS: LayerStruct, F, elem_step, layer_sbs, lo_ap, hi_ap, consume_chunk, dt_):
                # F: gathered row width (lhsT free dim); elem_step: table row stride
                idx_sb, wcol_sb, nval_sb = layer_sbs
                ramp32, ramp128 = ramps[dt_]
                for g_i, g in enumerate(LS.groups):
                    fb = g["first_blk"]
                    nlo, nhi = g["lo_cnt"], g["hi_cnt"]
                    msg_tiles = {}
                    for h, cnt_, table_ap in ((0, nlo, lo_ap), (1, nhi, hi_ap)):
                        if cnt_ == 0:
                            continue
                        nidx = cnt_ * BLK
                        col0 = (fb + (nlo if h else 0)) * 8
                        msg = msgp.tile([128, cnt_ * F], dt_, tag=f"msg_{h}")
                        if "nogather" in probe:
                            nc.vector.memset(msg[:, :1], 0.0)
                        else:
                            dma_gather_raw(
                                nc.gpsimd,
                                msg[:].rearrange("p (b f) -> p b f", b=cnt_),
                                table_ap,
                                idx_sb[:, col0 : col0 + nidx // 16],
                                num_idxs=nidx,
                                num_idxs_reg=nidx,
                                elem_size=F,
                                elem_step=elem_step,
                                single_packet=False,
                            )
                        msg_tiles[h] = msg
                    # ---- build ind on DVE: fulls then tails ----
                    nf, nt = g["nf"], g["nt"]
                    fbc, tbc = g["first_bc"], g["tail_bc0"]
                    ind_f = indp.tile([128, max(nf, 1) * W], dt_, tag="indf")
                    if nf and "noind" not in probe:
                        pred = (
                            ramp32[:]
                            .rearrange("p (o w) -> p o w", o=1)
                            .broadcast_to([128, nf, W])
                        )
                        wc = (
                            wcol_sb[:, fbc : fbc + nf]
                            .rearrange("p (b o) -> p b o", o=1)
                            .broadcast_to([128, nf, W])
                        )
                        nv = (
                            nval_sb[:, fbc : fbc + nf]
                            .rearrange("p (b o) -> p b o", o=1)
                            .broadcast_to([128, nf, W])
                        )
                        nc.vector.tensor_tensor(
                            out=ind_f[:].rearrange("p (b w) -> p b w", b=nf),
                            in0=pred, in1=wc, op=mybir.AluOpType.is_equal,
                        )
                        nc.vector.tensor_tensor(
                            out=ind_f[:].rearrange("p (b w) -> p b w", b=nf),
                            in0=ind_f[:].rearrange("p (b w) -> p b w", b=nf),
                            in1=nv, op=mybir.AluOpType.mult,
                        )
                    elif nf:
                        nc.vector.memset(ind_f[:, :1], 0.0)
                    ind_t = indp.tile([128, max(nt, 1) * BLK], dt_, tag="indt")
                    if nt and "noind" not in probe:
                        pred = (
                            ramp128[:]
                            .rearrange("p (o w) -> p o w", o=1)
                            .broadcast_to([128, nt, BLK])
                        )
                        wc = (
                            wcol_sb[:, tbc : tbc + nt]
                            .rearrange("p (b o) -> p b o", o=1)
                            .broadcast_to([128, nt, BLK])
                        )
                        nv = (
                            nval_sb[:, tbc : tbc + nt]
                            .rearrange("p (b o) -> p b o", o=1)
                            .broadcast_to([128, nt, BLK])
                        )
                        nc.vector.tensor_tensor(
                            out=ind_t[:].rearrange("p (b w) -> p b w", b=nt),
                            in0=pred, in1=wc, op=mybir.AluOpType.is_equal,
                        )
                        nc.vector.tensor_tensor(
                            out=ind_t[:].rearrange("p (b w) -> p b w", b=nt),
                            in0=ind_t[:].rearrange("p (b w) -> p b w", b=nt),
                            in1=nv, op=mybir.AluOpType.mult,
                        )
                    elif nt:
                        nc.vector.memset(ind_t[:, :1], 0.0)
                    for dep in (*msg_tiles.values(), ind_f, ind_t):
                        if "noabsorb" in probe:
                            break
                        absorb(dep[:, :1])
                    for kk in g["chunks"]:
                        blocks = LS.chunk_blocks[kk]
                        zps = zpsp.tile([128, 128], f32, space="PSUM", tag="z")
                        nc.tensor.matmul(
                            zps[:F, :], lhsT=zero_bf[:, :F], rhs=zero_bf[:, :],
                            start=True, stop=False,
                        )
                        for bi, (h, cs, kind, ric, width, ooff) in enumerate(blocks):
                            if "noblocks" in probe:
                                break
                            last = bi == len(blocks) - 1
                            msg = msg_tiles[h]
                            rhs_tile = ind_f if kind == "full" else ind_t
                            nc.tensor.matmul(
                                zps[:F, ooff : ooff + width],
                                lhsT=msg[:, cs * F : (cs + 1) * F],
                                rhs=rhs_tile[:, ric : ric + width],
                                start=False,
                                stop=last,
                            )
                        z_sb = sbp.tile([F, 128], f32, tag="z_sb")
                        nc.scalar.copy(z_sb[:], zps[:F, :])
                        consume_chunk(kk, z_sb)

            # ---- Layer 1 ----
            def l1_chunk(kk, z_sb):
                absorb(z_sb[:, :1])
                hps = hpsp.tile([H, 128], f32, space="PSUM", tag="h")
                nc.tensor.matmul(hps[:], lhsT=W1_sb[:], rhs=z_sb[:FIN, :], start=True, stop=True)
                h1T = sbp.tile([H, 128], f32, tag="h1T")
                nc.scalar.activation(
                    h1T[:], hps[:], mybir.ActivationFunctionType.Relu, bias=b1_sb[:, :]
                )
                absorb(h1T[:, :1])
                tps = tpsp.tile([128, H], f32, space="PSUM", tag="t")
                nc.tensor.transpose(out=tps[:], in_=h1T[:], identity=ident[:])
                h1n = sbp.tile([128, H], bf16, tag="h1n")
                nc.vector.tensor_copy(out=h1n[:], in_=tps[:])
                nc.sync.dma_start(out=h1sh[kk * 128 : (kk + 1) * 128, :], in_=h1n[:])

            l1_sbs = load_layer_inputs(L1, idx1_t, wcol1_t, nval1_t, "1", bf16)
            sparse_layer(
                L1, FIN, 128, l1_sbs,
                xbffull[0 : L1.n_lo_rows, 0:FIN],
                xbffull[L1.n_lo_rows : N, 0:FIN] if L1.n_hi_rows else xbffull[0:1, 0:FIN],
                l1_chunk,
                bf16,
            )

            # ---- AllGather h1 ----
            if single_core:
                nc.sync.dma_start(out=h1full[0:PADN, :], in_=h1sh[:, :])
            else:
                nc.gpsimd.collective_compute(
                    "AllGather",
                    mybir.AluOpType.bypass,
                    replica_groups=[list(range(NC))],
                    ins=[h1sh[:, :]],
                    outs=[h1full[:, :]],
                )

            # ---- Layer 2 + head ----
            def l2_chunk(kk, z_sb):
                absorb(z_sb[:, :1])
                h2T_halves = []
                for half_i in range(2):
                    hps = hpsp.tile([H, 128], f32, space="PSUM", tag="h")
                    nc.tensor.matmul(
                        hps[:],
                        lhsT=W2_sb[:, half_i * H : (half_i + 1) * H],
                        rhs=z_sb[:],
                        start=True,
                        stop=True,
                    )
                    h2T = sbp.tile([H, 128], f32, tag=f"h2T{half_i}")
                    nc.scalar.activation(
                        h2T[:],
                        hps[:],
                        mybir.ActivationFunctionType.Relu,
                        bias=b2_sb[:, half_i : half_i + 1],
                    )
                    h2T_halves.append(h2T)
                absorb(h2T_halves[0][:, :1])
                absorb(h2T_halves[1][:, :1])
                qps = qpsp.tile([128, 1], f32, space="PSUM", tag="q")
                for half_i in range(2):
                    nc.tensor.matmul(
                        qps[:],
                        lhsT=h2T_halves[half_i][:],
                        rhs=w3_sb[:, half_i : half_i + 1],
                        start=half_i == 0,
                        stop=half_i == 1,
                    )
                nc.vector.tensor_copy(out=q_sb[:, kk : kk + 1], in_=qps[:])
                nc.tensor.matmul(
                    pool_ps[:],
                    lhsT=Call[:, kk * G : (kk + 1) * G],
                    rhs=q_sb[:, kk : kk + 1],
                    start=kk == 0,
                    stop=kk == CH - 1,
                )

            l2_sbs = load_layer_inputs(L2, idx2_t, wcol2_t, nval2_t, "2", bf16)
            sparse_layer(
                L2, H, H, l2_sbs,
                h1full[0 : L2.n_lo_rows, :],
                h1full[L2.n_lo_rows : NC * PADN, :] if L2.n_hi_rows else h1full[0:1, :],
                l2_chunk,
                bf16,
            )

            pool_sb = sbp.tile([G, 1], f32, tag="pool")
            nc.vector.tensor_copy(out=pool_sb[:], in_=pool_ps[:])
            nc.sync.dma_start(out=out_t[:, :], in_=pool_sb[:])

    nc.compile()
    return nc


def postprocess(cfg: Cfg, results, host):
    out = np.zeros((cfg.G, 1), dtype=np.float64)
    for r in results:
        out += r["out"].astype(np.float64)
    out += host["c_const"]
    out[host["empty"], 0] = host["linb"]
    return out.astype(np.float32)


from concourse import bass_utils as _bass_utils


def kernel(**inputs) -> np.ndarray:
    cfg = Cfg()
    L1, L2, in_maps, host = preprocess(cfg, inputs)
    nc = build_module(cfg, L1, L2)
    res = _bass_utils.run_bass_kernel_spmd(nc, in_maps, core_ids=list(range(cfg.NC)))
    return postprocess(cfg, res.results, host)


# revision 5
# speedup vs baseline: 1.0353x; 1.0353x over previous
"""GCN (3-layer + mean-pool head) on 8 Trainium2 cores — v3 = v2 + bf16 L2 path.

bf16: h1 table (halves L2 gather traffic + h1 AllGather), L2 msg/ind matmuls
(4x PE stream rate vs fp32), C matrix + q (halves head DMA). L1 stays fp32.
"""

_V2_DOC = """GCN (3-layer + mean-pool head) on 8 Trainium2 cores — v2, slim inputs.

Differences from v1:
  - ind matrices built ON DEVICE from per-slot (wcol, norm) arrays via
    iota-ramp is_equal + multiply (upload 1.2MB/layer instead of 30MB).
  - idx uploaded un-tiled [16, TOT*8] and replicated to 128 partitions on
    device (0.3MB instead of 2.4MB per layer).
  - x uploaded sharded [NPC, FIN] per core and AllGathered on device
    (1.6MB instead of 12.8MB per core).
Per-group block enumeration: gather order per half = fulls then tails;
ind columns: fulls region (width 32 each) then tails region (width 128).
"""  # noqa: E501

from dataclasses import dataclass
import numpy as np

import concourse.bass as bass
import concourse.bacc as bacc
import concourse.mybir as mybir
import concourse.tile as tile
from concourse import ap_utils
from concourse._compat import exact_div
from concourse.masks import make_identity


def dma_gather_raw(gp, out_ap, in_ap, idxs_ap, num_idxs, num_idxs_reg, elem_size,
                   elem_step, single_packet=False):
    """BassGpSimd.dma_gather with the elem-size assert relaxed to 128B.

    The ISA encodes the table ROW STRIDE in 256-byte units
    (stride_bytes_256); the gathered element itself may be 128B —
    verified correct on HW (bench_elem128.py). Lets L1 gather 64 bf16
    features from a [N, 128] bf16 (256B-stride) table."""
    assert idxs_ap.dtype == mybir.dt.int16
    assert in_ap.space == bass.MemorySpace.DRAM
    assert idxs_ap.space == bass.MemorySpace.SBUF
    assert out_ap.space == bass.MemorySpace.SBUF
    assert in_ap.dtype == out_ap.dtype
    elem_size_bytes = elem_size * mybir.dt.size(in_ap.dtype)
    assert elem_size_bytes % 128 == 0
    assert ap_utils.ap_is_contiguous(in_ap.ap[1:])
    assert ap_utils.ap_is_contiguous(out_ap.ap[1:])
    assert ap_utils.ap_is_contiguous(idxs_ap.ap[1:])
    assert in_ap.ap[-1][1] == out_ap.ap[-1][1] == elem_size
    assert out_ap.ap[0][1] * out_ap.ap[1][1] == -(-num_idxs // 128) * 128
    assert in_ap.ap[0][0] == elem_step
    stride_bytes = elem_step * mybir.dt.size(in_ap.dtype)
    stride_bytes_256 = exact_div(stride_bytes, 256)
    assert stride_bytes_256 < 256
    _in_ap = gp.lower_ap_dma(in_ap, for_custom_bir_dma=True)
    _idxs_ap = gp.lower_ap(idxs_ap)
    _out_ap = gp.lower_ap(out_ap)
    return gp.add_instruction(
        mybir.InstDMAGatherAnt(
            name=gp.bass.get_next_instruction_name(),
            ins=[*_in_ap, _idxs_ap, gp.lower_val_access(gp.to_reg(num_idxs_reg))],
            outs=[_out_ap],
            transpose=False,
            num_idxs=num_idxs,
            elem_size=elem_size,
            stride_bytes_256=stride_bytes_256,
            gen_mode=0,
            single_packet=single_packet,
            queue_num=0,
            sbuf_tokens_per_rank=0,
            sbuf_free_dim_per_rank=0,
            sbuf_free_dim_pad_per_rank=0,
            sbuf_byte_offset=0,
        )
    )

BLK = 128
W = 32
NW = 4


@dataclass
class Cfg:
    N: int = 50000
    E: int = 1000000
    G: int = 128
    FIN: int = 64
    H: int = 128
    H2: int = 256
    NC: int = 8
    CG: int = 4
    SPLIT: int = 32768

    @property
    def NPC(self):
        assert self.N % self.NC == 0
        return self.N // self.NC

    @property
    def CH(self):
        return (self.NPC + 127) // 128

    @property
    def PADN(self):
        return self.CH * 128

    @property
    def NG(self):
        return (self.CH + self.CG - 1) // self.CG


def _ceil_div(a, b):
    return -(-a // b)


class LayerStruct:
    """Block structure shared across cores + per-core compact arrays.

    Per group g (CG chunks):
      gather order: half h: [fulls(k asc, j asc, b), tails(k asc, b)] -> cs
      ind columns:  fulls region [fulls h0 ++ fulls h1] (width 32 each),
                    tails region [tails h0 ++ tails h1] (width 128 each)
      idx16 columns: group base gcol0 = first_blk*8; h0 blocks then h1 blocks
        in gather order, 8 int16 cols per block.
    Per-core arrays:
      idx16 [16, TOT*8]   wrapped gather indices (block-major in gather order)
      wcol  [128, TOT_ind] f32 window col per slot (ind order: per group fulls
                           then tails, concatenated over groups)
      nval  [128, TOT_ind] f32 norm per slot (0 padding)
    """

    def __init__(self, cfg: Cfg, rows, dst, norm, n_table_rows):
        NC, CH, NPC, CG, SPLIT = cfg.NC, cfg.CH, cfg.NPC, cfg.CG, cfg.SPLIT
        core = dst // NPC
        l = dst - core * NPC
        k = l >> 7
        j = (l >> 5) & 3
        w32 = l & 31
        w128 = l & 127
        half = (rows >= SPLIT).astype(np.int64)
        self.n_lo_rows = min(SPLIT, n_table_rows)
        self.n_hi_rows = max(0, n_table_rows - SPLIT)

        key = (((core * CH + k) * 2 + half) * NW + j)
        counts = np.bincount(key, minlength=NC * CH * 2 * NW).reshape(NC, CH, 2, NW)
        Bfull = (counts // BLK).max(axis=0)  # [CH, 2, NW]
        leftover = counts - np.minimum(counts, Bfull[None] * BLK)
        tail_cnt = leftover.sum(axis=3)  # [NC, CH, 2]
        Btail = _ceil_div(tail_cnt, BLK).max(axis=0)  # [CH, 2]
        self.Bfull, self.Btail = Bfull, Btail

        # --- enumerate blocks ---
        # per (k,h,j): gather cs base; per (k,h): tail cs base
        # per block: ind column offset (fulls then tails region per group)
        full_cs = np.zeros((CH, 2, NW), dtype=np.int64)  # cs of first full blk
        tail_cs = np.zeros((CH, 2), dtype=np.int64)
        full_sg = np.zeros((CH, 2, NW), dtype=np.int64)  # global gather slot base
        tail_sg = np.zeros((CH, 2), dtype=np.int64)
        full_ic = np.zeros((CH, 2, NW), dtype=np.int64)  # ind col offset (global)
        tail_ic = np.zeros((CH, 2), dtype=np.int64)
        # ind-order column index (into wcol/nval [*, TOT_ind]) per block
        full_bc = np.zeros((CH, 2, NW), dtype=np.int64)
        tail_bc = np.zeros((CH, 2), dtype=np.int64)

        self.groups = []
        self.chunk_blocks = [None] * CH  # list of (h, cs, ric_kind, roff, width, ooff)
        cur_blk = 0  # global block counter (gather order, h-grouped per group)
        cur_ic = 0  # global ind col counter
        cur_bc = 0  # global ind-order block col counter
        for g in range(cfg.NG):
            ks = list(range(g * CG, min((g + 1) * CG, CH)))
            first_blk = cur_blk
            # gather order per half
            half_cnt = [0, 0]
            for h in (0, 1):
                cs = 0
                for kk in ks:
                    for jj in range(NW):
                        full_cs[kk, h, jj] = cs
                        cs += Bfull[kk, h, jj]
                for kk in ks:
                    tail_cs[kk, h] = cs
                    cs += Btail[kk, h]
                half_cnt[h] = cs
            nlo, nhi = half_cnt
            for h in (0, 1):
                base = first_blk + (nlo if h else 0)
                for kk in ks:
                    for jj in range(NW):
                        full_sg[kk, h, jj] = (base + full_cs[kk, h, jj]) * BLK
                    tail_sg[kk, h] = (base + tail_cs[kk, h]) * BLK
            # ind columns: fulls h0 ++ fulls h1, then tails h0 ++ tails h1
            first_ic = cur_ic
            first_bc = cur_bc
            nf = 0
            for h in (0, 1):
                for kk in ks:
                    for jj in range(NW):
                        full_ic[kk, h, jj] = cur_ic
                        full_bc[kk, h, jj] = cur_bc
                        cur_ic += Bfull[kk, h, jj] * W
                        cur_bc += Bfull[kk, h, jj]
                        nf += Bfull[kk, h, jj]
            ic_tail0 = cur_ic
            bc_tail0 = cur_bc
            nt = 0
            for h in (0, 1):
                for kk in ks:
                    tail_ic[kk, h] = cur_ic
                    tail_bc[kk, h] = cur_bc
                    cur_ic += Btail[kk, h] * BLK
                    cur_bc += Btail[kk, h]
                    nt += Btail[kk, h]
            cur_blk += nlo + nhi
            self.groups.append(
                dict(
                    chunks=ks,
                    first_blk=first_blk,
                    lo_cnt=nlo,
                    hi_cnt=nhi,
                    nf=nf,
                    nt=nt,
                    first_ic=first_ic,  # fulls ind region start (global col)
                    tail_ic0=ic_tail0,  # tails ind region start
                    first_bc=first_bc,  # fulls block-col start in wcol/nval
                    tail_bc0=bc_tail0,
                )
            )
            # per-chunk emission metadata
            for kk in ks:
                bl = []
                for h in (0, 1):
                    for jj in range(NW):
                        for b in range(Bfull[kk, h, jj]):
                            cs = full_cs[kk, h, jj] + b
                            ric = full_ic[kk, h, jj] + b * W - first_ic
                            bl.append((h, cs, "full", ric, W, jj * W))
                    for b in range(Btail[kk, h]):
                        cs = tail_cs[kk, h] + b
                        ric = tail_ic[kk, h] + b * BLK - ic_tail0
                        bl.append((h, cs, "tail", ric, BLK, 0))
                self.chunk_blocks[kk] = bl
        self.TOT = cur_blk
        self.IND_COLS = cur_ic
        self.TOTB = cur_bc  # == TOT

        # --- vectorized edge -> (slot, block) assignment ---
        order = np.lexsort((j, key))
        sk = key[order]
        newgrp = np.ones(len(sk), dtype=bool)
        newgrp[1:] = sk[1:] != sk[:-1]
        starts = np.flatnonzero(newgrp)
        lengths = np.diff(np.append(starts, len(sk)))
        rank_sorted = np.arange(len(sk)) - np.repeat(starts, lengths)
        rank = np.empty(len(sk), dtype=np.int64)
        rank[order] = rank_sorted  # rank within (core,k,half,j)

        capacity = Bfull[k, half, j] * BLK
        is_full = rank < capacity
        lo_pref = np.cumsum(leftover, axis=3) - leftover
        tail_rank = lo_pref[core, k, half, j] + (rank - capacity)

        # gather slot (s_global into idx16)
        sg_full = full_sg[k, half, j] + rank
        sg_tail = tail_sg[k, half] + tail_rank
        sg = np.where(is_full, sg_full, sg_tail)
        slot = np.where(is_full, rank % BLK, tail_rank % BLK)
        # ind-order block col (into wcol/nval) and window col
        bc_full = full_bc[k, half, j] + rank // BLK
        bc_tail_ = tail_bc[k, half] + tail_rank // BLK
        bc = np.where(is_full, bc_full, bc_tail_)
        wc = np.where(is_full, w32, w128)

        self.per_core = []
        for c in range(NC):
            m = core == c
            ncols = self.TOT * BLK // 16
            idx16 = np.zeros((16, ncols), dtype=np.int16)
            sgm = sg[m]
            vals = (rows[m] - half[m] * SPLIT).astype(np.int16)
            idx16[sgm % 16, sgm // 16] = vals
            wcol = np.zeros((BLK, self.TOTB), dtype=np.float32)
            nval = np.zeros((BLK, self.TOTB), dtype=np.float32)
            wcol[slot[m], bc[m]] = wc[m].astype(np.float32)
            nval[slot[m], bc[m]] = norm[m]
            self.per_core.append((idx16, wcol, nval))


def preprocess(cfg: Cfg, inputs):
    x = np.asarray(inputs["x"], dtype=np.float32)
    ei = np.asarray(inputs["edge_index"], dtype=np.int64)
    batch = np.asarray(inputs["batch"], dtype=np.int64)
    W1 = np.asarray(inputs["W1"], np.float32)
    b1 = np.asarray(inputs["b1"], np.float32)
    W2 = np.asarray(inputs["W2"], np.float32)
    b2 = np.asarray(inputs["b2"], np.float32)
    W3 = np.asarray(inputs["W3"], np.float32)
    b3 = np.asarray(inputs["b3"], np.float32)
    linW = np.asarray(inputs["linW"], np.float32)
    linb = np.asarray(inputs["linb"], np.float32)

    N, NC, NPC, PADN, CH, G = cfg.N, cfg.NC, cfg.NPC, cfg.PADN, cfg.CH, cfg.G
    src = np.concatenate([ei[0], np.arange(N, dtype=np.int64)])
    dst = np.concatenate([ei[1], np.arange(N, dtype=np.int64)])
    deg = np.bincount(dst, minlength=N).astype(np.float32)
    dinv = 1.0 / np.sqrt(deg)
    norm = (dinv[src] * dinv[dst]).astype(np.float32)

    # self edges (incl. random src==dst edges) are handled densely per chunk:
    # their message is the locally-resident row scaled by selfw = sum of norms
    nonself = src != dst
    src_e, dst_e, norm_e = src[nonself], dst[nonself], norm[nonself]
    selfw = np.bincount(
        dst[~nonself], weights=norm[~nonself].astype(np.float64), minlength=N
    ).astype(np.float32)

    L1 = LayerStruct(cfg, src_e, dst_e, norm_e, n_table_rows=N)
    r_of = (src_e // NPC) * PADN + (src_e % NPC)
    L2 = LayerStruct(cfg, r_of, dst_e, norm_e, n_table_rows=NC * PADN)

    cnt = np.maximum(np.bincount(batch, minlength=G), 1).astype(np.float32)
    coef = norm / cnt[batch[dst]]
    c_src = src // NPC
    kk = (src % NPC) >> 7
    ll = (src % NPC) & 127
    gg = batch[dst]
    flat = ((c_src * CH + kk) * 128 + ll) * G + gg
    C = np.bincount(flat, weights=coef.astype(np.float64), minlength=NC * CH * 128 * G)
    C = C.reshape(NC, CH * 128, G).astype(np.float32)

    w3 = (W3 @ linW).astype(np.float32)
    c_const = float(b3 @ linW[:, 0] + linb[0])
    empty = np.bincount(batch, minlength=G) == 0

    H = cfg.H
    bfnp = mybir.dt.np(mybir.dt.bfloat16)
    in_maps = []
    for c in range(NC):
        idx1, wcol1, nval1 = L1.per_core[c]
        idx2, wcol2, nval2 = L2.per_core[c]
        in_maps.append(
            {
                "xsh": x[c * NPC : (c + 1) * NPC, :].copy(),
                "W1": W1,
                "b1": b1.reshape(H, 1),
                "W2": W2,
                "b2": b2.reshape(2, H).T.copy(),
                "w3": w3.reshape(2, H).T.copy(),
                "idx1": idx1,
                "wcol1": wcol1.astype(bfnp),
                "nval1": nval1.astype(bfnp),
                "idx2": idx2,
                "wcol2": wcol2.astype(bfnp),
                "nval2": nval2.astype(bfnp),
                "C": C[c].astype(bfnp),
                "selfw": np.pad(
                    selfw[c * NPC : (c + 1) * NPC], (0, PADN - NPC)
                ).reshape(CH, 128).T.copy().astype(bfnp),
            }
        )
    host = dict(c_const=c_const, empty=empty, linb=float(linb[0]))
    return L1, L2, in_maps, host


def build_module(cfg: Cfg, L1: LayerStruct, L2: LayerStruct, single_core: bool = False, probe: str = ""):
    N, NC, NPC, PADN, CH, G = cfg.N, cfg.NC, cfg.NPC, cfg.PADN, cfg.CH, cfg.G
    FIN, H, H2 = cfg.FIN, cfg.H, cfg.H2
    f32 = mybir.dt.float32
    bf16 = mybir.dt.bfloat16
    i16 = mybir.dt.int16

    nc = bacc.Bacc("TRN2", debug=False, num_devices=1 if single_core else NC)
    xsh_t = nc.dram_tensor("xsh", [NPC, FIN], f32, kind="ExternalInput")
    W1_t = nc.dram_tensor("W1", [FIN, H], f32, kind="ExternalInput")
    b1_t = nc.dram_tensor("b1", [H, 1], f32, kind="ExternalInput")
    W2_t = nc.dram_tensor("W2", [H, H2], f32, kind="ExternalInput")
    b2_t = nc.dram_tensor("b2", [H, 2], f32, kind="ExternalInput")
    w3_t = nc.dram_tensor("w3", [H, 2], f32, kind="ExternalInput")
    idx1_t = nc.dram_tensor("idx1", [16, L1.TOT * 8], i16, kind="ExternalInput")
    wcol1_t = nc.dram_tensor("wcol1", [128, L1.TOTB], bf16, kind="ExternalInput")
    nval1_t = nc.dram_tensor("nval1", [128, L1.TOTB], bf16, kind="ExternalInput")
    idx2_t = nc.dram_tensor("idx2", [16, L2.TOT * 8], i16, kind="ExternalInput")
    wcol2_t = nc.dram_tensor("wcol2", [128, L2.TOTB], bf16, kind="ExternalInput")
    nval2_t = nc.dram_tensor("nval2", [128, L2.TOTB], bf16, kind="ExternalInput")
    C_t = nc.dram_tensor("C", [CH * 128, G], bf16, kind="ExternalInput")
    selfw_t = nc.dram_tensor("selfw", [128, CH], bf16, kind="ExternalInput")
    out_t = nc.dram_tensor("out", [G, 1], f32, kind="ExternalOutput")

    # bf16 x table padded to 128 cols (gather elem must be a multiple of 256B)
    xbfloc = nc.dram_tensor("xbfloc", [NPC, 128], bf16)
    xbffull = nc.dram_tensor("xbffull", [N, 128], bf16, addr_space="Shared")
    h1sh = nc.dram_tensor("h1sh", [PADN, H], bf16)
    h1full = nc.dram_tensor("h1full", [NC * PADN, H], bf16, addr_space="Shared")

    with tile.TileContext(nc) as tc:
        with (
            tc.tile_pool(name="const", bufs=1) as cpool,
            tc.tile_pool(name="idxall", bufs=1) as idxallp,
            tc.tile_pool(name="slotd", bufs=1) as slotp,
            tc.tile_pool(name="ind", bufs=2) as indp,
            tc.tile_pool(name="msg", bufs=2) as msgp,
            tc.tile_pool(name="sb", bufs=2) as sbp,
            tc.tile_pool(name="qpool", bufs=1) as qpool,
            tc.tile_pool(name="zps", bufs=2, space="PSUM") as zpsp,
            tc.tile_pool(name="hps", bufs=2, space="PSUM") as hpsp,
            tc.tile_pool(name="tps", bufs=1, space="PSUM") as tpsp,
            tc.tile_pool(name="qps", bufs=1, space="PSUM") as qpsp,
            tc.tile_pool(name="pps", bufs=1, space="PSUM") as ppsp,
            tc.tile_pool(name="scr", bufs=1, space="PSUM") as scrp,
        ):
            zero_sb = cpool.tile([128, 128], f32)
            nc.vector.memset(zero_sb[:], 0.0)
            zero_bf = cpool.tile([128, 128], bf16)
            nc.vector.memset(zero_bf[:], 0.0)
            ident = cpool.tile([128, 128], f32)
            make_identity(nc, ident[:])
            identb = cpool.tile([128, 128], bf16)
            make_identity(nc, identb[:])
            selfw_sb = cpool.tile([128, CH], bf16)
            nc.sync.dma_start(out=selfw_sb[:], in_=selfw_t[:, :])
            W1_sb = cpool.tile([FIN, H], f32)
            nc.sync.dma_start(out=W1_sb[:], in_=W1_t[:, :])
            b1_sb = cpool.tile([H, 1], f32)
            nc.sync.dma_start(out=b1_sb[:], in_=b1_t[:, :])
            W2_sb = cpool.tile([H, H2], f32)
            nc.sync.dma_start(out=W2_sb[:], in_=W2_t[:, :])
            b2_sb = cpool.tile([H, 2], f32)
            nc.sync.dma_start(out=b2_sb[:], in_=b2_t[:, :])
            w3_sb = cpool.tile([H, 2], f32)
            nc.sync.dma_start(out=w3_sb[:], in_=w3_t[:, :])
            scr_ps = scrp.tile([1, 1], f32, space="PSUM")
            q_sb = qpool.tile([128, CH], bf16)
            pool_ps = ppsp.tile([G, 1], f32, space="PSUM")
            # whole C matrix resident: one DMA instead of 49 small ones
            Call = qpool.tile([128, CH * G], bf16, name="Call")
            nc.sync.dma_start(
                out=Call[:].rearrange("p (k g) -> p k g", k=CH),
                in_=C_t[:, :].rearrange("(k p) g -> p k g", k=CH),
            )

            # iota ramps 0..31 and 0..127 (f32, same value in every partition)
            ramps = {}
            for dt_, dname in ((f32, "f"), (bf16, "b")):
                r32 = cpool.tile([128, W], dt_, name=f"ramp32{dname}")
                nc.gpsimd.iota(
                    r32[:], [[1, W]], channel_multiplier=0,
                    allow_small_or_imprecise_dtypes=True,
                )
                r128 = cpool.tile([128, BLK], dt_, name=f"ramp128{dname}")
                nc.gpsimd.iota(
                    r128[:], [[1, BLK]], channel_multiplier=0,
                    allow_small_or_imprecise_dtypes=True,
                )
                ramps[dt_] = (r32, r128)

            def absorb(dep_ap):
                kdim = dep_ap.shape[0]
                zt = zero_bf if dep_ap.dtype == bf16 else zero_sb
                nc.tensor.matmul(
                    scr_ps[:], lhsT=zt[:kdim, :1], rhs=dep_ap, start=True, stop=True
                )

            absorb(zero_sb[:, :1])
            for cst in (ident, identb, selfw_sb, W1_sb, b1_sb, W2_sb, b2_sb, w3_sb):
                absorb(cst[:, :1])
            act_scr = cpool.tile([H, 3], f32)
            nc.scalar.copy(act_scr[:, 0:1], b1_sb[:, :1])
            nc.scalar.copy(act_scr[:, 1:2], b2_sb[:, 0:1])
            nc.scalar.copy(act_scr[:, 2:3], b2_sb[:, 1:2])
            absorb(Call[:, :1])

            # ---- convert own x shard to bf16 padded [NPC, 128], AllGather ----
            # (collectives cannot read IO tensors, so the staging through
            # xbfloc also satisfies that rule)
            # bulk part: 6144 rows = 128 partitions x 48 rows, contiguous per
            # partition on both DRAM sides -> two large efficient DMAs
            RPP = NPC // 128  # 48 rows per partition
            NB = RPP * 128
            xs_cv = cpool.tile([128, RPP * FIN], f32, name="xs_cv")
            nc.sync.dma_start(
                out=xs_cv[:].rearrange("p (r f) -> p r f", f=FIN),
                in_=xsh_t[0:NB, :].rearrange("(p r) f -> p r f", r=RPP),
            )
            xcv = cpool.tile([128, RPP * 128], bf16, name="xcv")
            nc.vector.memset(xcv[:], 0.0)
            nc.vector.tensor_copy(
                out=xcv[:].rearrange("p (r k) -> p r k", k=128)[:, :, 0:FIN],
                in_=xs_cv[:].rearrange("p (r f) -> p r f", f=FIN),
            )
            nc.sync.dma_start(
                out=xbfloc[0:NB, :].rearrange("(p r) k -> p r k", r=RPP),
                in_=xcv[:].rearrange("p (r k) -> p r k", k=128),
            )
            # tail rows
            ntail = NPC - NB
            if ntail:
                xst = cpool.tile([128, FIN], f32, name="xst")
                nc.sync.dma_start(out=xst[:ntail, :], in_=xsh_t[NB:NPC, :])
                xcvt = cpool.tile([128, 128], bf16, name="xcvt")
                nc.vector.memset(xcvt[:], 0.0)
                nc.vector.tensor_copy(out=xcvt[:ntail, :FIN], in_=xst[:ntail, :])
                nc.sync.dma_start(out=xbfloc[NB:NPC, :], in_=xcvt[:ntail, :])
            if single_core:
                nc.sync.dma_start(out=xbffull[0:NPC, :], in_=xbfloc[:, :])
            else:
                nc.gpsimd.collective_compute(
                    "AllGather",
                    mybir.AluOpType.bypass,
                    replica_groups=[list(range(NC))],
                    ins=[xbfloc[:, :]],
                    outs=[xbffull[:, :]],
                )

            # ---- per-layer slot data ----
            def load_layer_inputs(LS, idx_t, wcol_t, nval_t, tag, dt_):
                idx_sb = idxallp.tile([128, LS.TOT * 8], i16, tag=f"idx{tag}")
                for r in range(8):
                    nc.sync.dma_start(
                        out=idx_sb[16 * r : 16 * (r + 1), :], in_=idx_t[:, :]
                    )
                wcol_sb = slotp.tile([128, LS.TOTB], dt_, tag=f"wc{tag}")
                nc.sync.dma_start(out=wcol_sb[:], in_=wcol_t[:, :])
                nval_sb = slotp.tile([128, LS.TOTB], dt_, tag=f"nv{tag}")
                nc.sync.dma_start(out=nval_sb[:], in_=nval_t[:, :])
                return idx_sb, wcol_sb, nval_sb

            def sparse_layer(LS: LayerStruct, F, elem_step, layer_sbs, lo_ap, hi_ap,
                             consume_chunk, dt_, self_src, self_rows):
                # F: gathered row width (lhsT free dim); elem_step: table row stride
                # self_src: local DRAM table holding this core's own rows (self
                # loops are a dense scaled-identity block, no gather needed)
                idx_sb, wcol_sb, nval_sb = layer_sbs
                ramp32, ramp128 = ramps[dt_]
                # all self rows, scaled, in one bulk load + one DVE pass
                ssraw = slotp.tile([128, CH * F], dt_, tag="ssraw")
                nb_self = self_rows // 128  # full chunks
                nc.sync.dma_start(
                    out=ssraw[:].rearrange("p (k f) -> p k f", f=F)[:, 0:nb_self, :],
                    in_=self_src[0 : nb_self * 128, 0:F].rearrange(
                        "(k p) f -> p k f", p=128
                    ),
                )
                if nb_self < CH:
                    ntl = self_rows - nb_self * 128
                    nc.vector.memset(ssraw[:, nb_self * F :], 0.0)
                    nc.sync.dma_start(
                        out=ssraw[:ntl, nb_self * F : nb_self * F + F],
                        in_=self_src[nb_self * 128 : self_rows, 0:F],
                    )
                ssall = slotp.tile([128, CH * F], dt_, tag="ssall")
                nc.vector.tensor_tensor(
                    out=ssall[:].rearrange("p (k f) -> p k f", f=F),
                    in0=ssraw[:].rearrange("p (k f) -> p k f", f=F),
                    in1=selfw_sb[:, :]
                    .rearrange("p (k o) -> p k o", o=1)
                    .broadcast_to([128, CH, F]),
                    op=mybir.AluOpType.mult,
                )
                absorb(ssall[:, :1])
                for g_i, g in enumerate(LS.groups):
                    fb = g["first_blk"]
                    nlo, nhi = g["lo_cnt"], g["hi_cnt"]
                    msg_tiles = {}
                    for h, cnt_, table_ap in ((0, nlo, lo_ap), (1, nhi, hi_ap)):
                        if cnt_ == 0:
                            continue
                        nidx = cnt_ * BLK
                        col0 = (fb + (nlo if h else 0)) * 8
                        msg = msgp.tile([128, cnt_ * F], dt_, tag=f"msg_{h}")
                        if "nogather" in probe:
                            nc.vector.memset(msg[:, :1], 0.0)
                        else:
                            dma_gather_raw(
                                nc.gpsimd,
                                msg[:].rearrange("p (b f) -> p b f", b=cnt_),
                                table_ap,
                                idx_sb[:, col0 : col0 + nidx // 16],
                                num_idxs=nidx,
                                num_idxs_reg=nidx,
                                elem_size=F,
                                elem_step=elem_step,
                                single_packet=False,
                            )
                        msg_tiles[h] = msg
                    # ---- build ind on DVE: fulls then tails ----
                    nf, nt = g["nf"], g["nt"]
                    fbc, tbc = g["first_bc"], g["tail_bc0"]
                    ind_f = indp.tile([128, max(nf, 1) * W], dt_, tag="indf")
                    if nf and "noind" not in probe:
                        pred = (
                            ramp32[:]
                            .rearrange("p (o w) -> p o w", o=1)
                            .broadcast_to([128, nf, W])
                        )
                        wc = (
                            wcol_sb[:, fbc : fbc + nf]
                            .rearrange("p (b o) -> p b o", o=1)
                            .broadcast_to([128, nf, W])
                        )
                        nv = (
                            nval_sb[:, fbc : fbc + nf]
                            .rearrange("p (b o) -> p b o", o=1)
                            .broadcast_to([128, nf, W])
                        )
                        nc.vector.tensor_tensor(
                            out=ind_f[:].rearrange("p (b w) -> p b w", b=nf),
                            in0=pred, in1=wc, op=mybir.AluOpType.is_equal,
                        )
                        nc.vector.tensor_tensor(
                            out=ind_f[:].rearrange("p (b w) -> p b w", b=nf),
                            in0=ind_f[:].rearrange("p (b w) -> p b w", b=nf),
                            in1=nv, op=mybir.AluOpType.mult,
                        )
                    elif nf:
                        nc.vector.memset(ind_f[:, :1], 0.0)
                    ind_t = indp.tile([128, max(nt, 1) * BLK], dt_, tag="indt")
                    if nt and "noind" not in probe:
                        pred = (
                            ramp128[:]
                            .rearrange("p (o w) -> p o w", o=1)
                            .broadcast_to([128, nt, BLK])
                        )
                        wc = (
                            wcol_sb[:, tbc : tbc + nt]
                            .rearrange("p (b o) -> p b o", o=1)
                            .broadcast_to([128, nt, BLK])
                        )
                        nv = (
                            nval_sb[:, tbc : tbc + nt]
                            .rearrange("p (b o) -> p b o", o=1)
                            .broadcast_to([128, nt, BLK])
                        )
                        nc.vector.tensor_tensor(
                            out=ind_t[:].rearrange("p (b w) -> p b w", b=nt),
                            in0=pred, in1=wc, op=mybir.AluOpType.is_equal,
                        )
                        nc.vector.tensor_tensor(
                            out=ind_t[:].rearrange("p (b w) -> p b w", b=nt),
                            in0=ind_t[:].rearrange("p (b w) -> p b w", b=nt),
                            in1=nv, op=mybir.AluOpType.mult,
                        )
                    elif nt:
                        nc.vector.memset(ind_t[:, :1], 0.0)
                    for dep in (*msg_tiles.values(), ind_f, ind_t):
                        if "noabsorb" in probe:
                            break
                        absorb(dep[:, :1])
                    for kk in g["chunks"]:
                        blocks = LS.chunk_blocks[kk]
                        zps = zpsp.tile([128, 128], f32, space="PSUM", tag="z")
                        nc.tensor.matmul(
                            zps[:F, :], lhsT=zero_bf[:, :F], rhs=zero_bf[:, :],
                            start=True, stop=False,
                        )
                        for bi, (h, cs, kind, ric, width, ooff) in enumerate(blocks):
                            if "noblocks" in probe:
                                break
                            msg = msg_tiles[h]
                            rhs_tile = ind_f if kind == "full" else ind_t
                            nc.tensor.matmul(
                                zps[:F, ooff : ooff + width],
                                lhsT=msg[:, cs * F : (cs + 1) * F],
                                rhs=rhs_tile[:, ric : ric + width],
                                start=False,
                                stop=False,
                            )
                        # dense self-loop block: z += (selfw ⊙ own_rows)^T
                        nc.tensor.matmul(
                            zps[:F, :],
                            lhsT=ssall[:, kk * F : (kk + 1) * F],
                            rhs=identb[:, :],
                            start=False,
                            stop=True,
                        )
                        z_sb = sbp.tile([F, 128], f32, tag="z_sb")
                        nc.scalar.copy(z_sb[:], zps[:F, :])
                        consume_chunk(kk, z_sb)

            # ---- Layer 1 ----
            def l1_chunk(kk, z_sb):
                absorb(z_sb[:, :1])
                hps = hpsp.tile([H, 128], f32, space="PSUM", tag="h")
                nc.tensor.matmul(hps[:], lhsT=W1_sb[:], rhs=z_sb[:FIN, :], start=True, stop=True)
                h1T = sbp.tile([H, 128], f32, tag="h1T")
                nc.scalar.activation(
                    h1T[:], hps[:], mybir.ActivationFunctionType.Relu, bias=b1_sb[:, :]
                )
                absorb(h1T[:, :1])
                tps = tpsp.tile([128, H], f32, space="PSUM", tag="t")
                nc.tensor.transpose(out=tps[:], in_=h1T[:], identity=ident[:])
                h1n = sbp.tile([128, H], bf16, tag="h1n")
                nc.vector.tensor_copy(out=h1n[:], in_=tps[:])
                nc.sync.dma_start(out=h1sh[kk * 128 : (kk + 1) * 128, :], in_=h1n[:])

            l1_sbs = load_layer_inputs(L1, idx1_t, wcol1_t, nval1_t, "1", bf16)
            sparse_layer(
                L1, FIN, 128, l1_sbs,
                xbffull[0 : L1.n_lo_rows, 0:FIN],
                xbffull[L1.n_lo_rows : N, 0:FIN] if L1.n_hi_rows else xbffull[0:1, 0:FIN],
                l1_chunk,
                bf16,
                xbfloc,
                NPC,
            )

            # ---- AllGather h1 ----
            if single_core:
                nc.sync.dma_start(out=h1full[0:PADN, :], in_=h1sh[:, :])
            else:
                nc.gpsimd.collective_compute(
                    "AllGather",
                    mybir.AluOpType.bypass,
                    replica_groups=[list(range(NC))],
                    ins=[h1sh[:, :]],
                    outs=[h1full[:, :]],
                )

            # ---- Layer 2 + head ----
            def l2_chunk(kk, z_sb):
                absorb(z_sb[:, :1])
                h2T_halves = []
                for half_i in range(2):
                    hps = hpsp.tile([H, 128], f32, space="PSUM", tag="h")
                    nc.tensor.matmul(
                        hps[:],
                        lhsT=W2_sb[:, half_i * H : (half_i + 1) * H],
                        rhs=z_sb[:],
                        start=True,
                        stop=True,
                    )
                    h2T = sbp.tile([H, 128], f32, tag=f"h2T{half_i}")
                    nc.scalar.activation(
                        h2T[:],
                        hps[:],
                        mybir.ActivationFunctionType.Relu,
                        bias=b2_sb[:, half_i : half_i + 1],
                    )
                    h2T_halves.append(h2T)
                absorb(h2T_halves[0][:, :1])
                absorb(h2T_halves[1][:, :1])
                qps = qpsp.tile([128, 1], f32, space="PSUM", tag="q")
                for half_i in range(2):
                    nc.tensor.matmul(
                        qps[:],
                        lhsT=h2T_halves[half_i][:],
                        rhs=w3_sb[:, half_i : half_i + 1],
                        start=half_i == 0,
                        stop=half_i == 1,
                    )
                nc.vector.tensor_copy(out=q_sb[:, kk : kk + 1], in_=qps[:])
                nc.tensor.matmul(
                    pool_ps[:],
                    lhsT=Call[:, kk * G : (kk + 1) * G],
                    rhs=q_sb[:, kk : kk + 1],
                    start=kk == 0,
                    stop=kk == CH - 1,
                )

            l2_sbs = load_layer_inputs(L2, idx2_t, wcol2_t, nval2_t, "2", bf16)
            sparse_layer(
                L2, H, H, l2_sbs,
                h1full[0 : L2.n_lo_rows, :],
                h1full[L2.n_lo_rows : NC * PADN, :] if L2.n_hi_rows else h1full[0:1, :],
                l2_chunk,
                bf16,
                h1sh,
                PADN,
            )

            pool_sb = sbp.tile([G, 1], f32, tag="pool")
            nc.vector.tensor_copy(out=pool_sb[:], in_=pool_ps[:])
            nc.sync.dma_start(out=out_t[:, :], in_=pool_sb[:])

    nc.compile()
    return nc


def postprocess(cfg: Cfg, results, host):
    out = np.zeros((cfg.G, 1), dtype=np.float64)
    for r in results:
        out += r["out"].astype(np.float64)
    out += host["c_const"]
    out[host["empty"], 0] = host["linb"]
    return out.astype(np.float32)


from concourse import bass_utils as _bass_utils


def kernel(**inputs) -> np.ndarray:
    cfg = Cfg()
    L1, L2, in_maps, host = preprocess(cfg, inputs)
    nc = build_module(cfg, L1, L2)
    res = _bass_utils.run_bass_kernel_spmd(nc, in_maps, core_ids=list(range(cfg.NC)))
    return postprocess(cfg, res.results, host)


# revision 6
# speedup vs baseline: 1.0419x; 1.0064x over previous
"""GCN (3-layer + mean-pool head) on 8 Trainium2 cores — v3 = v2 + bf16 L2 path.

bf16: h1 table (halves L2 gather traffic + h1 AllGather), L2 msg/ind matmuls
(4x PE stream rate vs fp32), C matrix + q (halves head DMA). L1 stays fp32.
"""

_V2_DOC = """GCN (3-layer + mean-pool head) on 8 Trainium2 cores — v2, slim inputs.

Differences from v1:
  - ind matrices built ON DEVICE from per-slot (wcol, norm) arrays via
    iota-ramp is_equal + multiply (upload 1.2MB/layer instead of 30MB).
  - idx uploaded un-tiled [16, TOT*8] and replicated to 128 partitions on
    device (0.3MB instead of 2.4MB per layer).
  - x uploaded sharded [NPC, FIN] per core and AllGathered on device
    (1.6MB instead of 12.8MB per core).
Per-group block enumeration: gather order per half = fulls then tails;
ind columns: fulls region (width 32 each) then tails region (width 128).
"""  # noqa: E501

from dataclasses import dataclass
import numpy as np

import concourse.bass as bass
import concourse.bacc as bacc
import concourse.mybir as mybir
import concourse.tile as tile
from concourse import ap_utils
from concourse._compat import exact_div
from concourse.masks import make_identity


def dma_gather_raw(gp, out_ap, in_ap, idxs_ap, num_idxs, num_idxs_reg, elem_size,
                   elem_step, single_packet=False):
    """BassGpSimd.dma_gather with the elem-size assert relaxed to 128B.

    The ISA encodes the table ROW STRIDE in 256-byte units
    (stride_bytes_256); the gathered element itself may be 128B —
    verified correct on HW (bench_elem128.py). Lets L1 gather 64 bf16
    features from a [N, 128] bf16 (256B-stride) table."""
    assert idxs_ap.dtype == mybir.dt.int16
    assert in_ap.space == bass.MemorySpace.DRAM
    assert idxs_ap.space == bass.MemorySpace.SBUF
    assert out_ap.space == bass.MemorySpace.SBUF
    assert in_ap.dtype == out_ap.dtype
    elem_size_bytes = elem_size * mybir.dt.size(in_ap.dtype)
    assert elem_size_bytes % 128 == 0
    assert ap_utils.ap_is_contiguous(in_ap.ap[1:])
    assert ap_utils.ap_is_contiguous(out_ap.ap[1:])
    assert ap_utils.ap_is_contiguous(idxs_ap.ap[1:])
    assert in_ap.ap[-1][1] == out_ap.ap[-1][1] == elem_size
    assert out_ap.ap[0][1] * out_ap.ap[1][1] == -(-num_idxs // 128) * 128
    assert in_ap.ap[0][0] == elem_step
    stride_bytes = elem_step * mybir.dt.size(in_ap.dtype)
    stride_bytes_256 = exact_div(stride_bytes, 256)
    assert stride_bytes_256 < 256
    _in_ap = gp.lower_ap_dma(in_ap, for_custom_bir_dma=True)
    _idxs_ap = gp.lower_ap(idxs_ap)
    _out_ap = gp.lower_ap(out_ap)
    return gp.add_instruction(
        mybir.InstDMAGatherAnt(
            name=gp.bass.get_next_instruction_name(),
            ins=[*_in_ap, _idxs_ap, gp.lower_val_access(gp.to_reg(num_idxs_reg))],
            outs=[_out_ap],
            transpose=False,
            num_idxs=num_idxs,
            elem_size=elem_size,
            stride_bytes_256=stride_bytes_256,
            gen_mode=0,
            single_packet=single_packet,
            queue_num=0,
            sbuf_tokens_per_rank=0,
            sbuf_free_dim_per_rank=0,
            sbuf_free_dim_pad_per_rank=0,
            sbuf_byte_offset=0,
        )
    )

BLK = 128
W = 32
NW = 4


@dataclass
class Cfg:
    N: int = 50000
    E: int = 1000000
    G: int = 128
    FIN: int = 64
    H: int = 128
    H2: int = 256
    NC: int = 8
    CG: int = 4
    SPLIT: int = 32768

    @property
    def NPC(self):
        assert self.N % self.NC == 0
        return self.N // self.NC

    @property
    def CH(self):
        return (self.NPC + 127) // 128

    @property
    def PADN(self):
        return self.CH * 128

    @property
    def NG(self):
        return (self.CH + self.CG - 1) // self.CG


def _ceil_div(a, b):
    return -(-a // b)


class LayerStruct:
    """Block structure shared across cores + per-core compact arrays.

    Per group g (CG chunks):
      gather order: half h: [fulls(k asc, j asc, b), tails(k asc, b)] -> cs
      ind columns:  fulls region [fulls h0 ++ fulls h1] (width 32 each),
                    tails region [tails h0 ++ tails h1] (width 128 each)
      idx16 columns: group base gcol0 = first_blk*8; h0 blocks then h1 blocks
        in gather order, 8 int16 cols per block.
    Per-core arrays:
      idx16 [16, TOT*8]   wrapped gather indices (block-major in gather order)
      wcol  [128, TOT_ind] f32 window col per slot (ind order: per group fulls
                           then tails, concatenated over groups)
      nval  [128, TOT_ind] f32 norm per slot (0 padding)
    """

    def __init__(self, cfg: Cfg, rows, dst, norm, n_table_rows, half, lo_boundary):
        # half: per-edge lo/hi bit (shared across layers so the block
        # structure is identical); lo_boundary: table row count of the lo half
        NC, CH, NPC, CG = cfg.NC, cfg.CH, cfg.NPC, cfg.CG
        SPLIT = lo_boundary
        core = dst // NPC
        l = dst - core * NPC
        k = l >> 7
        j = (l >> 5) & 3
        w32 = l & 31
        w128 = l & 127
        assert np.all((rows >= SPLIT) == (half == 1))
        self.n_lo_rows = min(SPLIT, n_table_rows)
        self.n_hi_rows = max(0, n_table_rows - SPLIT)

        key = (((core * CH + k) * 2 + half) * NW + j)
        counts = np.bincount(key, minlength=NC * CH * 2 * NW).reshape(NC, CH, 2, NW)
        Bfull = (counts // BLK).max(axis=0)  # [CH, 2, NW]
        leftover = counts - np.minimum(counts, Bfull[None] * BLK)
        tail_cnt = leftover.sum(axis=3)  # [NC, CH, 2]
        Btail = _ceil_div(tail_cnt, BLK).max(axis=0)  # [CH, 2]
        self.Bfull, self.Btail = Bfull, Btail

        # --- enumerate blocks ---
        # per (k,h,j): gather cs base; per (k,h): tail cs base
        # per block: ind column offset (fulls then tails region per group)
        full_cs = np.zeros((CH, 2, NW), dtype=np.int64)  # cs of first full blk
        tail_cs = np.zeros((CH, 2), dtype=np.int64)
        full_sg = np.zeros((CH, 2, NW), dtype=np.int64)  # global gather slot base
        tail_sg = np.zeros((CH, 2), dtype=np.int64)
        full_ic = np.zeros((CH, 2, NW), dtype=np.int64)  # ind col offset (global)
        tail_ic = np.zeros((CH, 2), dtype=np.int64)
        # ind-order column index (into wcol/nval [*, TOT_ind]) per block
        full_bc = np.zeros((CH, 2, NW), dtype=np.int64)
        tail_bc = np.zeros((CH, 2), dtype=np.int64)

        self.groups = []
        self.chunk_blocks = [None] * CH  # list of (h, cs, ric_kind, roff, width, ooff)
        cur_blk = 0  # global block counter (gather order, h-grouped per group)
        cur_ic = 0  # global ind col counter
        cur_bc = 0  # global ind-order block col counter
        for g in range(cfg.NG):
            ks = list(range(g * CG, min((g + 1) * CG, CH)))
            first_blk = cur_blk
            # gather order per half
            half_cnt = [0, 0]
            for h in (0, 1):
                cs = 0
                for kk in ks:
                    for jj in range(NW):
                        full_cs[kk, h, jj] = cs
                        cs += Bfull[kk, h, jj]
                for kk in ks:
                    tail_cs[kk, h] = cs
                    cs += Btail[kk, h]
                half_cnt[h] = cs
            nlo, nhi = half_cnt
            for h in (0, 1):
                base = first_blk + (nlo if h else 0)
                for kk in ks:
                    for jj in range(NW):
                        full_sg[kk, h, jj] = (base + full_cs[kk, h, jj]) * BLK
                    tail_sg[kk, h] = (base + tail_cs[kk, h]) * BLK
            # ind columns: fulls h0 ++ fulls h1, then tails h0 ++ tails h1
            first_ic = cur_ic
            first_bc = cur_bc
            nf = 0
            for h in (0, 1):
                for kk in ks:
                    for jj in range(NW):
                        full_ic[kk, h, jj] = cur_ic
                        full_bc[kk, h, jj] = cur_bc
                        cur_ic += Bfull[kk, h, jj] * W
                        cur_bc += Bfull[kk, h, jj]
                        nf += Bfull[kk, h, jj]
            ic_tail0 = cur_ic
            bc_tail0 = cur_bc
            nt = 0
            for h in (0, 1):
                for kk in ks:
                    tail_ic[kk, h] = cur_ic
                    tail_bc[kk, h] = cur_bc
                    cur_ic += Btail[kk, h] * BLK
                    cur_bc += Btail[kk, h]
                    nt += Btail[kk, h]
            cur_blk += nlo + nhi
            self.groups.append(
                dict(
                    chunks=ks,
                    first_blk=first_blk,
                    lo_cnt=nlo,
                    hi_cnt=nhi,
                    nf=nf,
                    nt=nt,
                    first_ic=first_ic,  # fulls ind region start (global col)
                    tail_ic0=ic_tail0,  # tails ind region start
                    first_bc=first_bc,  # fulls block-col start in wcol/nval
                    tail_bc0=bc_tail0,
                )
            )
            # per-chunk emission metadata
            for kk in ks:
                bl = []
                for h in (0, 1):
                    for jj in range(NW):
                        for b in range(Bfull[kk, h, jj]):
                            cs = full_cs[kk, h, jj] + b
                            ric = full_ic[kk, h, jj] + b * W - first_ic
                            bl.append((h, cs, "full", ric, W, jj * W))
                    for b in range(Btail[kk, h]):
                        cs = tail_cs[kk, h] + b
                        ric = tail_ic[kk, h] + b * BLK - ic_tail0
                        bl.append((h, cs, "tail", ric, BLK, 0))
                self.chunk_blocks[kk] = bl
        self.TOT = cur_blk
        self.IND_COLS = cur_ic
        self.TOTB = cur_bc  # == TOT

        # --- vectorized edge -> (slot, block) assignment ---
        order = np.lexsort((j, key))
        sk = key[order]
        newgrp = np.ones(len(sk), dtype=bool)
        newgrp[1:] = sk[1:] != sk[:-1]
        starts = np.flatnonzero(newgrp)
        lengths = np.diff(np.append(starts, len(sk)))
        rank_sorted = np.arange(len(sk)) - np.repeat(starts, lengths)
        rank = np.empty(len(sk), dtype=np.int64)
        rank[order] = rank_sorted  # rank within (core,k,half,j)

        capacity = Bfull[k, half, j] * BLK
        is_full = rank < capacity
        lo_pref = np.cumsum(leftover, axis=3) - leftover
        tail_rank = lo_pref[core, k, half, j] + (rank - capacity)

        # gather slot (s_global into idx16)
        sg_full = full_sg[k, half, j] + rank
        sg_tail = tail_sg[k, half] + tail_rank
        sg = np.where(is_full, sg_full, sg_tail)
        slot = np.where(is_full, rank % BLK, tail_rank % BLK)
        # ind-order block col (into wcol/nval) and window col
        bc_full = full_bc[k, half, j] + rank // BLK
        bc_tail_ = tail_bc[k, half] + tail_rank // BLK
        bc = np.where(is_full, bc_full, bc_tail_)
        wc = np.where(is_full, w32, w128)

        self.per_core = []
        for c in range(NC):
            m = core == c
            ncols = self.TOT * BLK // 16
            idx16 = np.zeros((16, ncols), dtype=np.int16)
            sgm = sg[m]
            vals = (rows[m] - half[m] * SPLIT).astype(np.int16)
            idx16[sgm % 16, sgm // 16] = vals
            wcol = np.zeros((BLK, self.TOTB), dtype=np.float32)
            nval = np.zeros((BLK, self.TOTB), dtype=np.float32)
            wcol[slot[m], bc[m]] = wc[m].astype(np.float32)
            nval[slot[m], bc[m]] = norm[m]
            self.per_core.append((idx16, wcol, nval))


def preprocess(cfg: Cfg, inputs):
    x = np.asarray(inputs["x"], dtype=np.float32)
    ei = np.asarray(inputs["edge_index"], dtype=np.int64)
    batch = np.asarray(inputs["batch"], dtype=np.int64)
    W1 = np.asarray(inputs["W1"], np.float32)
    b1 = np.asarray(inputs["b1"], np.float32)
    W2 = np.asarray(inputs["W2"], np.float32)
    b2 = np.asarray(inputs["b2"], np.float32)
    W3 = np.asarray(inputs["W3"], np.float32)
    b3 = np.asarray(inputs["b3"], np.float32)
    linW = np.asarray(inputs["linW"], np.float32)
    linb = np.asarray(inputs["linb"], np.float32)

    N, NC, NPC, PADN, CH, G = cfg.N, cfg.NC, cfg.NPC, cfg.PADN, cfg.CH, cfg.G
    src = np.concatenate([ei[0], np.arange(N, dtype=np.int64)])
    dst = np.concatenate([ei[1], np.arange(N, dtype=np.int64)])
    deg = np.bincount(dst, minlength=N).astype(np.float32)
    dinv = 1.0 / np.sqrt(deg)
    norm = (dinv[src] * dinv[dst]).astype(np.float32)

    # self edges (incl. random src==dst edges) are handled densely per chunk:
    # their message is the locally-resident row scaled by selfw = sum of norms
    nonself = src != dst
    src_e, dst_e, norm_e = src[nonself], dst[nonself], norm[nonself]
    selfw = np.bincount(
        dst[~nonself], weights=norm[~nonself].astype(np.float64), minlength=N
    ).astype(np.float32)

    # shared half split: choose B so both src<B and r_of<r_of(B) fit int16;
    # with identical half bits both layers get byte-identical block structure
    B = 32656
    B2 = (B // NPC) * PADN + (B % NPC)  # r_of(B) = 32766 < 32768
    half_e = (src_e >= B).astype(np.int64)
    L1 = LayerStruct(cfg, src_e, dst_e, norm_e, n_table_rows=N, half=half_e, lo_boundary=B)
    r_of = (src_e // NPC) * PADN + (src_e % NPC)
    L2 = LayerStruct(cfg, r_of, dst_e, norm_e, n_table_rows=NC * PADN, half=half_e, lo_boundary=B2)
    for c in range(NC):
        assert np.array_equal(L1.per_core[c][1], L2.per_core[c][1])  # wcol
        assert np.array_equal(L1.per_core[c][2], L2.per_core[c][2])  # nval

    cnt = np.maximum(np.bincount(batch, minlength=G), 1).astype(np.float32)
    coef = norm / cnt[batch[dst]]
    c_src = src // NPC
    kk = (src % NPC) >> 7
    ll = (src % NPC) & 127
    gg = batch[dst]
    flat = ((c_src * CH + kk) * 128 + ll) * G + gg
    C = np.bincount(flat, weights=coef.astype(np.float64), minlength=NC * CH * 128 * G)
    C = C.reshape(NC, CH * 128, G).astype(np.float32)

    w3 = (W3 @ linW).astype(np.float32)
    c_const = float(b3 @ linW[:, 0] + linb[0])
    empty = np.bincount(batch, minlength=G) == 0

    H = cfg.H
    bfnp = mybir.dt.np(mybir.dt.bfloat16)
    in_maps = []
    for c in range(NC):
        idx1, wcol1, nval1 = L1.per_core[c]
        idx2 = L2.per_core[c][0]
        in_maps.append(
            {
                "xsh": x[c * NPC : (c + 1) * NPC, :].copy(),
                "W1": W1,
                "b1": b1.reshape(H, 1),
                "W2": W2,
                "b2": b2.reshape(2, H).T.copy(),
                "w3": w3.reshape(2, H).T.copy(),
                "idx1": idx1,
                "wcol": wcol1.astype(bfnp),
                "nval": nval1.astype(bfnp),
                "idx2": idx2,
                "C": C[c].astype(bfnp),
                "selfw": np.pad(
                    selfw[c * NPC : (c + 1) * NPC], (0, PADN - NPC)
                ).reshape(CH, 128).T.copy().astype(bfnp),
            }
        )
    host = dict(c_const=c_const, empty=empty, linb=float(linb[0]))
    return L1, L2, in_maps, host


def build_module(cfg: Cfg, L1: LayerStruct, L2: LayerStruct, single_core: bool = False, probe: str = ""):
    N, NC, NPC, PADN, CH, G = cfg.N, cfg.NC, cfg.NPC, cfg.PADN, cfg.CH, cfg.G
    FIN, H, H2 = cfg.FIN, cfg.H, cfg.H2
    f32 = mybir.dt.float32
    bf16 = mybir.dt.bfloat16
    i16 = mybir.dt.int16

    nc = bacc.Bacc("TRN2", debug=False, num_devices=1 if single_core else NC)
    xsh_t = nc.dram_tensor("xsh", [NPC, FIN], f32, kind="ExternalInput")
    W1_t = nc.dram_tensor("W1", [FIN, H], f32, kind="ExternalInput")
    b1_t = nc.dram_tensor("b1", [H, 1], f32, kind="ExternalInput")
    W2_t = nc.dram_tensor("W2", [H, H2], f32, kind="ExternalInput")
    b2_t = nc.dram_tensor("b2", [H, 2], f32, kind="ExternalInput")
    w3_t = nc.dram_tensor("w3", [H, 2], f32, kind="ExternalInput")
    idx1_t = nc.dram_tensor("idx1", [16, L1.TOT * 8], i16, kind="ExternalInput")
    wcol_t = nc.dram_tensor("wcol", [128, L1.TOTB], bf16, kind="ExternalInput")
    nval_t = nc.dram_tensor("nval", [128, L1.TOTB], bf16, kind="ExternalInput")
    idx2_t = nc.dram_tensor("idx2", [16, L2.TOT * 8], i16, kind="ExternalInput")
    C_t = nc.dram_tensor("C", [CH * 128, G], bf16, kind="ExternalInput")
    selfw_t = nc.dram_tensor("selfw", [128, CH], bf16, kind="ExternalInput")
    out_t = nc.dram_tensor("out", [G, 1], f32, kind="ExternalOutput")

    # bf16 x table padded to 128 cols (gather elem must be a multiple of 256B)
    xbfloc = nc.dram_tensor("xbfloc", [NPC, 128], bf16)
    xbffull = nc.dram_tensor("xbffull", [N, 128], bf16, addr_space="Shared")
    h1sh = nc.dram_tensor("h1sh", [PADN, H], bf16)
    h1full = nc.dram_tensor("h1full", [NC * PADN, H], bf16, addr_space="Shared")

    with tile.TileContext(nc) as tc:
        with (
            tc.tile_pool(name="const", bufs=1) as cpool,
            tc.tile_pool(name="idxall", bufs=1) as idxallp,
            tc.tile_pool(name="slotd", bufs=1) as slotp,
            tc.tile_pool(name="ind", bufs=2) as indp,
            tc.tile_pool(name="msg", bufs=2) as msgp,
            tc.tile_pool(name="sb", bufs=2) as sbp,
            tc.tile_pool(name="qpool", bufs=1) as qpool,
            tc.tile_pool(name="zps", bufs=2, space="PSUM") as zpsp,
            tc.tile_pool(name="hps", bufs=2, space="PSUM") as hpsp,
            tc.tile_pool(name="tps", bufs=1, space="PSUM") as tpsp,
            tc.tile_pool(name="qps", bufs=1, space="PSUM") as qpsp,
            tc.tile_pool(name="pps", bufs=1, space="PSUM") as ppsp,
            tc.tile_pool(name="scr", bufs=1, space="PSUM") as scrp,
        ):
            zero_sb = cpool.tile([128, 128], f32)
            nc.vector.memset(zero_sb[:], 0.0)
            zero_bf = cpool.tile([128, 128], bf16)
            nc.vector.memset(zero_bf[:], 0.0)
            ident = cpool.tile([128, 128], f32)
            make_identity(nc, ident[:])
            identb = cpool.tile([128, 128], bf16)
            make_identity(nc, identb[:])
            selfw_sb = cpool.tile([128, CH], bf16)
            nc.sync.dma_start(out=selfw_sb[:], in_=selfw_t[:, :])
            W1_sb = cpool.tile([FIN, H], f32)
            nc.sync.dma_start(out=W1_sb[:], in_=W1_t[:, :])
            b1_sb = cpool.tile([H, 1], f32)
            nc.sync.dma_start(out=b1_sb[:], in_=b1_t[:, :])
            W2_sb = cpool.tile([H, H2], f32)
            nc.sync.dma_start(out=W2_sb[:], in_=W2_t[:, :])
            b2_sb = cpool.tile([H, 2], f32)
            nc.sync.dma_start(out=b2_sb[:], in_=b2_t[:, :])
            w3_sb = cpool.tile([H, 2], f32)
            nc.sync.dma_start(out=w3_sb[:], in_=w3_t[:, :])
            scr_ps = scrp.tile([1, 1], f32, space="PSUM")
            q_sb = qpool.tile([128, CH], bf16)
            pool_ps = ppsp.tile([G, 1], f32, space="PSUM")
            # whole C matrix resident: one DMA instead of 49 small ones
            Call = qpool.tile([128, CH * G], bf16, name="Call")
            nc.sync.dma_start(
                out=Call[:].rearrange("p (k g) -> p k g", k=CH),
                in_=C_t[:, :].rearrange("(k p) g -> p k g", k=CH),
            )

            # iota ramps 0..31 and 0..127 (f32, same value in every partition)
            ramps = {}
            for dt_, dname in ((f32, "f"), (bf16, "b")):
                r32 = cpool.tile([128, W], dt_, name=f"ramp32{dname}")
                nc.gpsimd.iota(
                    r32[:], [[1, W]], channel_multiplier=0,
                    allow_small_or_imprecise_dtypes=True,
                )
                r128 = cpool.tile([128, BLK], dt_, name=f"ramp128{dname}")
                nc.gpsimd.iota(
                    r128[:], [[1, BLK]], channel_multiplier=0,
                    allow_small_or_imprecise_dtypes=True,
                )
                ramps[dt_] = (r32, r128)

            def absorb(dep_ap):
                kdim = dep_ap.shape[0]
                zt = zero_bf if dep_ap.dtype == bf16 else zero_sb
                nc.tensor.matmul(
                    scr_ps[:], lhsT=zt[:kdim, :1], rhs=dep_ap, start=True, stop=True
                )

            absorb(zero_sb[:, :1])
            for cst in (ident, identb, selfw_sb, W1_sb, b1_sb, W2_sb, b2_sb, w3_sb):
                absorb(cst[:, :1])
            act_scr = cpool.tile([H, 3], f32)
            nc.scalar.copy(act_scr[:, 0:1], b1_sb[:, :1])
            nc.scalar.copy(act_scr[:, 1:2], b2_sb[:, 0:1])
            nc.scalar.copy(act_scr[:, 2:3], b2_sb[:, 1:2])
            absorb(Call[:, :1])

            # ---- convert own x shard to bf16 padded [NPC, 128], AllGather ----
            # (collectives cannot read IO tensors, so the staging through
            # xbfloc also satisfies that rule)
            # bulk part: 6144 rows = 128 partitions x 48 rows, contiguous per
            # partition on both DRAM sides -> two large efficient DMAs
            RPP = NPC // 128  # 48 rows per partition
            NB = RPP * 128
            xs_cv = cpool.tile([128, RPP * FIN], f32, name="xs_cv")
            nc.sync.dma_start(
                out=xs_cv[:].rearrange("p (r f) -> p r f", f=FIN),
                in_=xsh_t[0:NB, :].rearrange("(p r) f -> p r f", r=RPP),
            )
            xcv = cpool.tile([128, RPP * 128], bf16, name="xcv")
            nc.vector.memset(xcv[:], 0.0)
            nc.vector.tensor_copy(
                out=xcv[:].rearrange("p (r k) -> p r k", k=128)[:, :, 0:FIN],
                in_=xs_cv[:].rearrange("p (r f) -> p r f", f=FIN),
            )
            nc.sync.dma_start(
                out=xbfloc[0:NB, :].rearrange("(p r) k -> p r k", r=RPP),
                in_=xcv[:].rearrange("p (r k) -> p r k", k=128),
            )
            # tail rows
            ntail = NPC - NB
            if ntail:
                xst = cpool.tile([128, FIN], f32, name="xst")
                nc.sync.dma_start(out=xst[:ntail, :], in_=xsh_t[NB:NPC, :])
                xcvt = cpool.tile([128, 128], bf16, name="xcvt")
                nc.vector.memset(xcvt[:], 0.0)
                nc.vector.tensor_copy(out=xcvt[:ntail, :FIN], in_=xst[:ntail, :])
                nc.sync.dma_start(out=xbfloc[NB:NPC, :], in_=xcvt[:ntail, :])
            if single_core:
                nc.sync.dma_start(out=xbffull[0:NPC, :], in_=xbfloc[:, :])
            else:
                nc.gpsimd.collective_compute(
                    "AllGather",
                    mybir.AluOpType.bypass,
                    replica_groups=[list(range(NC))],
                    ins=[xbfloc[:, :]],
                    outs=[xbffull[:, :]],
                )

            # ---- per-layer slot data ----
            wcol_sb = slotp.tile([128, L1.TOTB], bf16, name="wcol_sb")
            nc.sync.dma_start(out=wcol_sb[:], in_=wcol_t[:, :])
            nval_sb = slotp.tile([128, L1.TOTB], bf16, name="nval_sb")
            nc.sync.dma_start(out=nval_sb[:], in_=nval_t[:, :])

            def load_layer_inputs(LS, idx_t, tag):
                idx_sb = idxallp.tile([128, LS.TOT * 8], i16, tag=f"idx{tag}")
                for r in range(8):
                    nc.sync.dma_start(
                        out=idx_sb[16 * r : 16 * (r + 1), :], in_=idx_t[:, :]
                    )
                return idx_sb, wcol_sb, nval_sb

            def sparse_layer(LS: LayerStruct, F, elem_step, layer_sbs, lo_ap, hi_ap,
                             consume_chunk, dt_, self_src, self_rows):
                # F: gathered row width (lhsT free dim); elem_step: table row stride
                # self_src: local DRAM table holding this core's own rows (self
                # loops are a dense scaled-identity block, no gather needed)
                idx_sb, wcol_sb, nval_sb = layer_sbs
                ramp32, ramp128 = ramps[dt_]
                # all self rows, scaled, in one bulk load + one DVE pass
                ssraw = slotp.tile([128, CH * F], dt_, tag="ssraw")
                nb_self = self_rows // 128  # full chunks
                nc.sync.dma_start(
                    out=ssraw[:].rearrange("p (k f) -> p k f", f=F)[:, 0:nb_self, :],
                    in_=self_src[0 : nb_self * 128, 0:F].rearrange(
                        "(k p) f -> p k f", p=128
                    ),
                )
                if nb_self < CH:
                    ntl = self_rows - nb_self * 128
                    nc.vector.memset(ssraw[:, nb_self * F :], 0.0)
                    nc.sync.dma_start(
                        out=ssraw[:ntl, nb_self * F : nb_self * F + F],
                        in_=self_src[nb_self * 128 : self_rows, 0:F],
                    )
                ssall = slotp.tile([128, CH * F], dt_, tag="ssall")
                nc.vector.tensor_tensor(
                    out=ssall[:].rearrange("p (k f) -> p k f", f=F),
                    in0=ssraw[:].rearrange("p (k f) -> p k f", f=F),
                    in1=selfw_sb[:, :]
                    .rearrange("p (k o) -> p k o", o=1)
                    .broadcast_to([128, CH, F]),
                    op=mybir.AluOpType.mult,
                )
                absorb(ssall[:, :1])
                for g_i, g in enumerate(LS.groups):
                    fb = g["first_blk"]
                    nlo, nhi = g["lo_cnt"], g["hi_cnt"]
                    msg_tiles = {}
                    for h, cnt_, table_ap in ((0, nlo, lo_ap), (1, nhi, hi_ap)):
                        if cnt_ == 0:
                            continue
                        nidx = cnt_ * BLK
                        col0 = (fb + (nlo if h else 0)) * 8
                        msg = msgp.tile([128, cnt_ * F], dt_, tag=f"msg_{h}")
                        if "nogather" in probe:
                            nc.vector.memset(msg[:, :1], 0.0)
                        else:
                            dma_gather_raw(
                                nc.gpsimd,
                                msg[:].rearrange("p (b f) -> p b f", b=cnt_),
                                table_ap,
                                idx_sb[:, col0 : col0 + nidx // 16],
                                num_idxs=nidx,
                                num_idxs_reg=nidx,
                                elem_size=F,
                                elem_step=elem_step,
                                single_packet=False,
                            )
                        msg_tiles[h] = msg
                    # ---- build ind on DVE: fulls then tails ----
                    nf, nt = g["nf"], g["nt"]
                    fbc, tbc = g["first_bc"], g["tail_bc0"]
                    ind_f = indp.tile([128, max(nf, 1) * W], dt_, tag="indf")
                    if nf and "noind" not in probe:
                        pred = (
                            ramp32[:]
                            .rearrange("p (o w) -> p o w", o=1)
                            .broadcast_to([128, nf, W])
                        )
                        wc = (
                            wcol_sb[:, fbc : fbc + nf]
                            .rearrange("p (b o) -> p b o", o=1)
                            .broadcast_to([128, nf, W])
                        )
                        nv = (
                            nval_sb[:, fbc : fbc + nf]
                            .rearrange("p (b o) -> p b o", o=1)
                            .broadcast_to([128, nf, W])
                        )
                        nc.vector.tensor_tensor(
                            out=ind_f[:].rearrange("p (b w) -> p b w", b=nf),
                            in0=pred, in1=wc, op=mybir.AluOpType.is_equal,
                        )
                        nc.vector.tensor_tensor(
                            out=ind_f[:].rearrange("p (b w) -> p b w", b=nf),
                            in0=ind_f[:].rearrange("p (b w) -> p b w", b=nf),
                            in1=nv, op=mybir.AluOpType.mult,
                        )
                    elif nf:
                        nc.vector.memset(ind_f[:, :1], 0.0)
                    ind_t = indp.tile([128, max(nt, 1) * BLK], dt_, tag="indt")
                    if nt and "noind" not in probe:
                        pred = (
                            ramp128[:]
                            .rearrange("p (o w) -> p o w", o=1)
                            .broadcast_to([128, nt, BLK])
                        )
                        wc = (
                            wcol_sb[:, tbc : tbc + nt]
                            .rearrange("p (b o) -> p b o", o=1)
                            .broadcast_to([128, nt, BLK])
                        )
                        nv = (
                            nval_sb[:, tbc : tbc + nt]
                            .rearrange("p (b o) -> p b o", o=1)
                            .broadcast_to([128, nt, BLK])
                        )
                        nc.vector.tensor_tensor(
                            out=ind_t[:].rearrange("p (b w) -> p b w", b=nt),
                            in0=pred, in1=wc, op=mybir.AluOpType.is_equal,
                        )
                        nc.vector.tensor_tensor(
                            out=ind_t[:].rearrange("p (b w) -> p b w", b=nt),
                            in0=ind_t[:].rearrange("p (b w) -> p b w", b=nt),
                            in1=nv, op=mybir.AluOpType.mult,
                        )
                    elif nt:
                        nc.vector.memset(ind_t[:, :1], 0.0)
                    for dep in (*msg_tiles.values(), ind_f, ind_t):
                        if "noabsorb" in probe:
                            break
                        absorb(dep[:, :1])
                    for kk in g["chunks"]:
                        blocks = LS.chunk_blocks[kk]
                        zps = zpsp.tile([128, 128], f32, space="PSUM", tag="z")
                        nc.tensor.matmul(
                            zps[:F, :], lhsT=zero_bf[:, :F], rhs=zero_bf[:, :],
                            start=True, stop=False,
                        )
                        for bi, (h, cs, kind, ric, width, ooff) in enumerate(blocks):
                            if "noblocks" in probe:
                                break
                            msg = msg_tiles[h]
                            rhs_tile = ind_f if kind == "full" else ind_t
                            nc.tensor.matmul(
                                zps[:F, ooff : ooff + width],
                                lhsT=msg[:, cs * F : (cs + 1) * F],
                                rhs=rhs_tile[:, ric : ric + width],
                                start=False,
                                stop=False,
                            )
                        # dense self-loop block: z += (selfw ⊙ own_rows)^T
                        nc.tensor.matmul(
                            zps[:F, :],
                            lhsT=ssall[:, kk * F : (kk + 1) * F],
                            rhs=identb[:, :],
                            start=False,
                            stop=True,
                        )
                        z_sb = sbp.tile([F, 128], f32, tag="z_sb")
                        nc.scalar.copy(z_sb[:], zps[:F, :])
                        consume_chunk(kk, z_sb)

            # ---- Layer 1 ----
            def l1_chunk(kk, z_sb):
                absorb(z_sb[:, :1])
                hps = hpsp.tile([H, 128], f32, space="PSUM", tag="h")
                nc.tensor.matmul(hps[:], lhsT=W1_sb[:], rhs=z_sb[:FIN, :], start=True, stop=True)
                h1T = sbp.tile([H, 128], f32, tag="h1T")
                nc.scalar.activation(
                    h1T[:], hps[:], mybir.ActivationFunctionType.Relu, bias=b1_sb[:, :]
                )
                absorb(h1T[:, :1])
                tps = tpsp.tile([128, H], f32, space="PSUM", tag="t")
                nc.tensor.transpose(out=tps[:], in_=h1T[:], identity=ident[:])
                h1n = sbp.tile([128, H], bf16, tag="h1n")
                nc.vector.tensor_copy(out=h1n[:], in_=tps[:])
                nc.sync.dma_start(out=h1sh[kk * 128 : (kk + 1) * 128, :], in_=h1n[:])

            l1_sbs = load_layer_inputs(L1, idx1_t, "1")
            sparse_layer(
                L1, FIN, 128, l1_sbs,
                xbffull[0 : L1.n_lo_rows, 0:FIN],
                xbffull[L1.n_lo_rows : N, 0:FIN] if L1.n_hi_rows else xbffull[0:1, 0:FIN],
                l1_chunk,
                bf16,
                xbfloc,
                NPC,
            )

            # ---- AllGather h1 ----
            if single_core:
                nc.sync.dma_start(out=h1full[0:PADN, :], in_=h1sh[:, :])
            else:
                nc.gpsimd.collective_compute(
                    "AllGather",
                    mybir.AluOpType.bypass,
                    replica_groups=[list(range(NC))],
                    ins=[h1sh[:, :]],
                    outs=[h1full[:, :]],
                )

            # ---- Layer 2 + head ----
            def l2_chunk(kk, z_sb):
                absorb(z_sb[:, :1])
                h2T_halves = []
                for half_i in range(2):
                    hps = hpsp.tile([H, 128], f32, space="PSUM", tag="h")
                    nc.tensor.matmul(
                        hps[:],
                        lhsT=W2_sb[:, half_i * H : (half_i + 1) * H],
                        rhs=z_sb[:],
                        start=True,
                        stop=True,
                    )
                    h2T = sbp.tile([H, 128], f32, tag=f"h2T{half_i}")
                    nc.scalar.activation(
                        h2T[:],
                        hps[:],
                        mybir.ActivationFunctionType.Relu,
                        bias=b2_sb[:, half_i : half_i + 1],
                    )
                    h2T_halves.append(h2T)
                absorb(h2T_halves[0][:, :1])
                absorb(h2T_halves[1][:, :1])
                qps = qpsp.tile([128, 1], f32, space="PSUM", tag="q")
                for half_i in range(2):
                    nc.tensor.matmul(
                        qps[:],
                        lhsT=h2T_halves[half_i][:],
                        rhs=w3_sb[:, half_i : half_i + 1],
                        start=half_i == 0,
                        stop=half_i == 1,
                    )
                nc.vector.tensor_copy(out=q_sb[:, kk : kk + 1], in_=qps[:])
                nc.tensor.matmul(
                    pool_ps[:],
                    lhsT=Call[:, kk * G : (kk + 1) * G],
                    rhs=q_sb[:, kk : kk + 1],
                    start=kk == 0,
                    stop=kk == CH - 1,
                )

            l2_sbs = load_layer_inputs(L2, idx2_t, "2")
            sparse_layer(
                L2, H, H, l2_sbs,
                h1full[0 : L2.n_lo_rows, :],
                h1full[L2.n_lo_rows : NC * PADN, :] if L2.n_hi_rows else h1full[0:1, :],
                l2_chunk,
                bf16,
                h1sh,
                PADN,
            )

            pool_sb = sbp.tile([G, 1], f32, tag="pool")
            nc.vector.tensor_copy(out=pool_sb[:], in_=pool_ps[:])
            nc.sync.dma_start(out=out_t[:, :], in_=pool_sb[:])

    nc.compile()
    return nc


def postprocess(cfg: Cfg, results, host):
    out = np.zeros((cfg.G, 1), dtype=np.float64)
    for r in results:
        out += r["out"].astype(np.float64)
    out += host["c_const"]
    out[host["empty"], 0] = host["linb"]
    return out.astype(np.float32)


from concourse import bass_utils as _bass_utils


def kernel(**inputs) -> np.ndarray:
    cfg = Cfg()
    L1, L2, in_maps, host = preprocess(cfg, inputs)
    nc = build_module(cfg, L1, L2)
    res = _bass_utils.run_bass_kernel_spmd(nc, in_maps, core_ids=list(range(cfg.NC)))
    return postprocess(cfg, res.results, host)
